# revision 15
# baseline (speedup 1.0000x reference)
"""Trainium2 Bass kernel for nn_DA_conv: per-sample generated depthwise 3x3 conv
-> relu -> 1x1 pointwise conv (+bias) -> + x * channel_attention(altitude).

Data-parallel over batch: 8 samples -> 8 NeuronCores, weights replicated.

v8 design (trace-driven, from the 80.8us v5 and the 100us v7 experiment):
  * DVE 4x tensor_scalar mode requires an even row count per op (free size
    multiple of 256 elements): v7's 13-row slices silently halved DVE tap
    throughput (1070ns vs 683ns per TS). All chunks use dve_rows=12.
  * Rows PE 80 / DVE 48 (v5: 84/44). Measured rates: PE row 484ns, DVE row
    (9 TS @4x + 8 TT @2x tree) ~1.09us incl per-op overhead.
  * First ~6us are framework boot (engine TENSOR_LOADs); no kernel
    instruction can run earlier, so preamble DMA order only matters after
    that. wblob_a goes first on the sync queue (feat matmul is the head of
    the dependency chain); xp0 pieces ride the scalar queue in parallel.
  * No activation-table preload: Sigmoid and Relu live in different tables
    (v7's preload cost 2x1283ns on the Act queue ahead of the critical
    wblob_a trigger).
  * TAIL_LAG=3 + pss 3 bufs: the PE keeps a chunk of depthwise work queued
    ahead of tails so it never starves on the DVE tree (v7 starved, and the
    HAM clock thrashed between full and half speed in 3.4us epochs).
  * Chunk 3 is flipped (DVE rows on top, PE rows at the bottom) so the last
    tails consume PE-produced rows; its tail evacs alternate vector/scalar
    to drain the endgame faster.
  * Residual x*att rides the pointwise PSUM group as a diag(att) matmul.
    (gpsimd cannot run TensorScalarPtr/TensorTensor: Pool engine check fails
    at codegen; DVE STT runs at 1x so it is slower than the TS+TT tree.)
"""

import os
from collections import deque
from contextlib import ExitStack

import ml_dtypes
import numpy as np

import concourse.bass as bass
import concourse.mybir as mybir
import concourse.tile as tile
from concourse import bacc
from concourse.bass_utils import run_bass_kernel_spmd

AF = mybir.ActivationFunctionType
ALU = mybir.AluOpType
F32 = mybir.dt.float32
BF16 = mybir.dt.bfloat16

B, C, H, W = 8, 128, 128, 128
KK = 3
NT = KK * KK                 # 9 taps
HW = H * W
XOFF = 2                     # interior column offset in the padded layout
WP = W + 4                   # host-padded width (2 left, 2 right)
HP = H + 2                   # host-padded height (1 halo row each side)
R = 32                       # image rows per chunk
NCH = H // R                 # 4 chunks
PE_ROWS_L = [18, 20, 20, 20]  # per-chunk TensorE depthwise rows (chunk-0 DVE part larger: DVE idles during the preamble anyway)
FLIP_LAST = True             # chunk 3: DVE rows on top, PE rows at the bottom
TAPS = [(dy, dx) for dy in (-1, 0, 1) for dx in (-1, 0, 1)]  # t = (dy+1)*3+(dx+1)
N_WARM = 3                   # PE warm-up matmuls ahead of the feat matmul

# bf16 weight blob a: w1t | alt | ca1t | ca2t   (small, lands first)
W_W1T, W_ALT, W_CA1T, W_CA2T = 0, 128, 129, 145
WA_COLS = 145 + 128
# bf16 weight blob b: w2t (tap-major: col t*128+c) | cwt
W_W2T, W_CWT = 0, NT * 128
WB_COLS = NT * 128 + 128
WB_SPLIT = 5 * 128           # w2t columns for taps 0-4 (first diag half)
MASK_SPLIT = 5 * 128         # mask columns for the first diag-build half

last_results = None          # BassKernelResults of the most recent run


def _pe_blocks(pe_lo, pe_hi):
    blocks = []
    r = pe_lo
    while r < pe_hi:
        rr = min(4, pe_hi - r)
        blocks.append((r, r + rr))
        r += rr
    return blocks


def _emit(tc, nc, d):
    ctx = d["ctx"]
    singles = ctx.enter_context(tc.tile_pool(name="singles", bufs=1))
    xpool = ctx.enter_context(tc.tile_pool(name="xpool", bufs=3))
    spool = ctx.enter_context(tc.tile_pool(name="spool", bufs=3))
    tpool = ctx.enter_context(tc.tile_pool(name="tpool", bufs=3))
    opool = ctx.enter_context(tc.tile_pool(name="opool", bufs=6))
    pss_pool = ctx.enter_context(tc.tile_pool(name="psum_s", bufs=2, space="PSUM"))
    pso_pool = ctx.enter_context(tc.tile_pool(name="psum_o", bufs=3, space="PSUM"))

    # -- a few PE warm-up matmuls (HAM ramp), then the preamble DMAs on the
    # sync queue in v5's split order: the first wblob_b/mask halves unlock
    # ktab taps 0-4 and the first dg half while the rest streams in --
    warm = singles.tile([128, 512], BF16, name="warm", tag="warm")
    nc.gpsimd.memset(warm, 0.0)
    wps = pso_pool.tile([128, 512], F32, name="wps", tag="pso")
    for _ in range(N_WARM):
        nc.tensor.matmul(wps, lhsT=warm[:, 0:128], rhs=warm, start=True, stop=True)

    wblob_a = singles.tile([128, WA_COLS], BF16, name="wblob_a", tag="wblob_a")
    nc.sync.dma_start(out=wblob_a, in_=d["wblob_a"])
    wblob_b = singles.tile([128, WB_COLS], BF16, name="wblob_b", tag="wblob_b")
    nc.sync.dma_start(out=wblob_b[:, 0:WB_SPLIT], in_=d["wblob_b"][:, 0:WB_SPLIT])
    mask = singles.tile([128, NT * 128], BF16, name="mask", tag="mask")
    nc.sync.dma_start(out=mask[:, 0:MASK_SPLIT], in_=d["mask"][:, 0:MASK_SPLIT])

    x3 = d["xpad"].rearrange("c (h w) -> c h w", w=WP)
    xpf_d = d["xpad"]

    xps = []
    xp0 = xpool.tile([128, R + 2, WP], BF16, name="xp0", tag="xp")
    nc.sync.dma_start(out=xp0[:, 0:6, :], in_=x3[:, 0:6, :])
    nc.sync.dma_start(
        out=wblob_b[:, WB_SPLIT:WB_COLS], in_=d["wblob_b"][:, WB_SPLIT:WB_COLS]
    )
    nc.sync.dma_start(
        out=mask[:, MASK_SPLIT : NT * 128], in_=d["mask"][:, MASK_SPLIT : NT * 128]
    )
    nc.sync.dma_start(out=xp0[:, 6:18, :], in_=x3[:, 6:18, :])
    cb = singles.tile([128, 1], F32, name="cb", tag="cb")
    nc.sync.dma_start(out=cb, in_=d["cb"])
    nc.sync.dma_start(out=xp0[:, 18 : R + 2, :], in_=x3[:, 18 : R + 2, :])
    xps.append(xp0)
    _emit_prologue(tc, nc, d, singles, pss_pool, wblob_a, wblob_b, mask,
                   warm, wps)
    for ci in range(1, NCH):
        y0 = ci * R
        xp = xpool.tile([128, R + 2, WP], BF16, name=f"xp{ci}", tag="xp")
        nc.sync.dma_start(out=xp, in_=x3[:, y0 : y0 + R + 2, :])
        xps.append(xp)

    ktab = d["ktab"]
    dg_all = d["dg_all"]
    cwt = wblob_b[:, W_CWT : W_CWT + 128]
    out_d = d["out"]

    # PE-row-gated tails flush as soon as emitted; DVE-row-gated tails are
    # held until the NEXT chunk's PE blocks are in the queue, so the PE always
    # has independent depthwise work ahead of a tail that may wait on the DVE
    # tree -> scalar relu chain (v11 lost ~5us to those stalls).
    held_dve_tails = []

    for ci in range(NCH):
        y0 = ci * R
        flip = FLIP_LAST and ci == NCH - 1
        pe_rows = PE_ROWS_L[ci]
        dve_rows = R - pe_rows
        dve_lo = 0 if flip else pe_rows
        dve_hi = dve_rows if flip else R
        pe_lo = dve_rows if flip else 0
        pe_hi = R if flip else pe_rows
        xp = xps[ci]
        srelu = spool.tile([128, R * W], BF16, name=f"sr{ci}", tag="sr")

        # -- DVE depthwise rows [dve_lo, dve_hi): 9 TS products into slots of
        # one 4D tile, then a contiguous-halves add tree (4 TTs, big APs) --
        P = tpool.tile([128, 8, dve_rows, W], BF16, name=f"P{ci}", tag="P")
        t8 = tpool.tile([128, dve_rows, W], BF16, name=f"t8{ci}", tag="t8")
        for ti, (dy, dx) in enumerate(TAPS):
            src = xp[:, 1 + dve_lo + dy : 1 + dve_hi + dy,
                     XOFF + dx : XOFF + dx + W]
            out = t8 if ti == 8 else P[:, ti]
            nc.vector.tensor_scalar_mul(out=out, in0=src,
                                        scalar1=ktab[:, ti : ti + 1])
        nc.vector.tensor_tensor(out=P[:, 0:4], in0=P[:, 0:4], in1=P[:, 4:8],
                                op=ALU.add)
        nc.vector.tensor_tensor(out=P[:, 0:2], in0=P[:, 0:2], in1=P[:, 2:4],
                                op=ALU.add)
        nc.vector.tensor_tensor(out=P[:, 0], in0=P[:, 0], in1=P[:, 1],
                                op=ALU.add)
        nc.vector.tensor_tensor(out=P[:, 0], in0=P[:, 0], in1=t8, op=ALU.add)
        nc.scalar.activation(srelu[:, dve_lo * W : dve_hi * W], P[:, 0],
                             AF.Relu)
        if ci == 0:
            d["emit_attention"]()
            attd = d["attd"]

        # -- PE depthwise: rows [pe_lo, pe_hi) in <=4-row PSUM blocks --
        for rs, re in _pe_blocks(pe_lo, pe_hi):
            rows = re - rs
            pss = pss_pool.tile([128, rows * W], F32, name=f"pss{ci}_{rs}",
                                tag="pss")
            for ti, (dy, dx) in enumerate(TAPS):
                rhs = xp[:, 1 + rs + dy : 1 + re + dy, XOFF + dx : XOFF + dx + W]
                nc.tensor.matmul(
                    pss, lhsT=dg_all[:, ti * 128 : (ti + 1) * 128], rhs=rhs,
                    start=(ti == 0), stop=(ti == NT - 1),
                )
            nc.scalar.activation(srelu[:, rs * W : re * W], pss, AF.Relu)

        # previous chunk's DVE-gated tails now have a chunk of PE blocks ahead
        for t in held_dve_tails:
            t()
        held_dve_tails = []

        # -- tails: 8-row units (2 PSUM banks of pointwise+residual, 1 store) --
        last = ci == NCH - 1
        for ti_, tr in enumerate(range(0, R, 8)):
            dve_gated = (tr + 8 > dve_lo) and (tr < dve_hi)
            evac_vec = last and (ti_ % 2 == 0)
            t = _make_tail(nc, pso_pool, opool, xp, srelu, cwt, attd,
                           cb, out_d, ci, tr, y0, evac_vec)
            if dve_gated and not last:
                held_dve_tails.append(t)
            else:
                t()
    for t in held_dve_tails:
        t()


def _emit_prologue(tc, nc, d, singles, pss_pool, wblob_a, wblob_b, mask,
                   warm, wps):
    def bridge(n):
        # fill PE-idle dependency stalls so the HAM clock never re-throttles
        for _ in range(n):
            nc.tensor.matmul(wps, lhsT=warm[:, 0:128], rhs=warm,
                             start=True, stop=True)

    alt = wblob_a[:, W_ALT : W_ALT + 1]
    w1t = wblob_a[:, W_W1T : W_W1T + 128]
    ca1t = wblob_a[:, W_CA1T : W_CA1T + 16]
    ca2t = wblob_a[0:16, W_CA2T : W_CA2T + 128]
    w2t = wblob_b[:, W_W2T : W_W2T + NT * 128]

    def leaky(name, psum_src, parts, dt=F32):
        tmp = singles.tile([parts, 1], F32, name=f"{name}_t", tag=f"{name}_t")
        nc.vector.tensor_scalar_mul(out=tmp, in0=psum_src, scalar1=0.1)
        res = singles.tile([parts, 1], dt, name=name, tag=name)
        nc.vector.tensor_tensor(out=res, in0=tmp, in1=psum_src, op=ALU.max)
        return res

    # ---- kernel-generator MLP (all bf16 matmuls) ----
    feat_ps = pss_pool.tile([128, 1], F32, name="feat_ps", tag="pss")
    nc.tensor.matmul(feat_ps, lhsT=w1t, rhs=alt, start=True, stop=True)
    bridge(2)
    feat = leaky("feat", feat_ps, 128, dt=BF16)

    ktab_ps = pss_pool.tile([128, NT], F32, name="ktab_ps", tag="pss")
    for t in range(5):
        nc.tensor.matmul(
            ktab_ps[:, t : t + 1], lhsT=w2t[:, t * 128 : (t + 1) * 128],
            rhs=feat, start=True, stop=True
        )
    bridge(1)
    for t in range(5, NT):
        nc.tensor.matmul(
            ktab_ps[:, t : t + 1], lhsT=w2t[:, t * 128 : (t + 1) * 128],
            rhs=feat, start=True, stop=True
        )
    bridge(2)
    # SBUF copy (DVE tap scalar source) on the vector engine, ahead of the
    # chunk-0 taps in its queue; the dg build reads ktab_ps straight from PSUM.
    ktab = singles.tile([128, NT], F32, name="ktab", tag="ktab")
    nc.vector.tensor_scalar_add(out=ktab, in0=ktab_ps, scalar1=0.0)

    # ---- diag weights straight from PSUM: dg[:, t*128+j] = I[p,j]*ktab[p,t]
    dg_all = singles.tile([128, NT * 128], BF16, name="dg_all", tag="dg_all")
    ktab_b = ktab_ps.unsqueeze(2).broadcast_to([128, NT, 128])
    mask3 = mask.rearrange("p (t c) -> p t c", t=NT)
    dg3 = dg_all.rearrange("p (t c) -> p t c", t=NT)
    nc.vector.tensor_tensor(
        out=dg3[:, 0:5, :], in0=mask3[:, 0:5, :], in1=ktab_b[:, 0:5, :],
        op=ALU.mult,
    )
    nc.vector.tensor_tensor(
        out=dg3[:, 5:NT, :], in0=mask3[:, 5:NT, :], in1=ktab_b[:, 5:NT, :],
        op=ALU.mult,
    )

    # keep the PE busy across the preamble->body handoff: an idle epoch here
    # makes the HAM down-throttle right as the first depthwise blocks issue
    # (the dg chain completes ~15us: boot+barrier+DMA flow are fixed costs)
    bridge(12)
    d["ktab"] = ktab
    d["dg_all"] = dg_all

    def emit_attention():
        # deferred until after chunk 0's tap emission: the Sigmoid act-table
        # load (~1.3us on the scalar engine) and the attd build must not gate
        # the first DVE taps; attd is first needed by the chunk-0 tails.
        a1_ps = pss_pool.tile([16, 1], F32, name="a1_ps", tag="pss")
        nc.tensor.matmul(a1_ps, lhsT=ca1t, rhs=alt, start=True, stop=True)
        a1 = leaky("a1", a1_ps, 16, dt=BF16)
        att_ps = pss_pool.tile([128, 1], F32, name="att_ps", tag="pss")
        nc.tensor.matmul(att_ps, lhsT=ca2t, rhs=a1, start=True, stop=True)
        attv = singles.tile([128, 1], F32, name="attv", tag="attv")
        nc.scalar.activation(attv, att_ps, AF.Sigmoid)
        attd = singles.tile([128, 128], BF16, name="attd", tag="attd")
        nc.vector.tensor_tensor(
            out=attd, in0=mask[:, 0:128], in1=attv.broadcast_to([128, 128]),
            op=ALU.mult,
        )
        d["attd"] = attd
    d["emit_attention"] = emit_attention


def _make_tail(nc, pso_pool, opool, xp, srelu, cwt, attd, cb, out_d, ci, tr,
               y0, evac_vec=False):
    """Two 4-row pointwise+residual groups into one 2-bank PSUM tile, single
    biased bf16 evac and store for chunk-relative rows [tr, tr+8)."""

    def tail():
        osb = opool.tile([128, 8 * W], BF16, name=f"ob{ci}_{tr}", tag="ob")
        pso = pso_pool.tile([128, 8 * W], F32, name=f"pso{ci}_{tr}", tag="pso")
        for h, r0 in enumerate((tr, tr + 4)):
            half = pso[:, h * 4 * W : (h + 1) * 4 * W]
            nc.tensor.matmul(half, lhsT=cwt, rhs=srelu[:, r0 * W : (r0 + 4) * W],
                             start=True, stop=False)
            nc.tensor.matmul(
                half, lhsT=attd, rhs=xp[:, 1 + r0 : 1 + r0 + 4, XOFF : XOFF + W],
                start=False, stop=True,
            )
        if evac_vec:
            nc.vector.tensor_scalar_add(out=osb, in0=pso, scalar1=cb)
        else:
            nc.scalar.activation(osb, pso, AF.Identity, bias=cb)
        nc.gpsimd.dma_start(
            out=out_d[:, (y0 + tr) * W : (y0 + tr + 8) * W], in_=osb
        )

    return tail


def build_module():
    nc = bacc.Bacc(
        "TRN2",
        target_bir_lowering=False,
        debug=False,
        enable_asserts=False,
        num_devices=B,
    )
    d = {
        "xpad": nc.dram_tensor("xpad", [C, HP * WP], BF16, kind="ExternalInput").ap(),
        "wblob_a": nc.dram_tensor("wblob_a", [128, WA_COLS], BF16, kind="ExternalInput").ap(),
        "wblob_b": nc.dram_tensor("wblob_b", [128, WB_COLS], BF16, kind="ExternalInput").ap(),
        "cb": nc.dram_tensor("cb", [C, 1], F32, kind="ExternalInput").ap(),
        "mask": nc.dram_tensor("mask", [128, NT * 128], BF16, kind="ExternalInput").ap(),
        "out": nc.dram_tensor("out", [C, HW], BF16, kind="ExternalOutput").ap(),
    }
    with tile.TileContext(nc) as tc:
        with ExitStack() as ctx:
            d["ctx"] = ctx
            _emit(tc, nc, d)
    nc.finalize()
    return nc


_module_cache = None


def _get_module():
    global _module_cache
    if _module_cache is None:
        _module_cache = build_module()
    return _module_cache


def make_in_maps(x, altitude, W1, W2, conv_w, conv_b, ca_w1, ca_w2):
    f = np.float32
    bf = ml_dtypes.bfloat16
    x = np.asarray(x, dtype=f)
    altitude = np.asarray(altitude, dtype=f)
    xpad = np.zeros((B, C, HP, WP), dtype=f)
    xpad[:, :, 1 : H + 1, XOFF : XOFF + W] = x
    xq = np.ascontiguousarray(xpad.astype(bf).reshape(B, C, HP * WP))

    wblob_shared = np.zeros((128, WA_COLS), dtype=bf)
    wblob_shared[:, W_W1T : W_W1T + 128] = np.asarray(W1, dtype=f).T.astype(bf)
    wblob_shared[:, W_CA1T : W_CA1T + 16] = np.asarray(ca_w1, dtype=f).T.astype(bf)
    wblob_shared[0:16, W_CA2T : W_CA2T + 128] = np.asarray(
        ca_w2, dtype=f
    ).T.astype(bf)
    wblob_b = np.zeros((128, WB_COLS), dtype=bf)
    w2tr = (
        np.asarray(W2, dtype=f).T.reshape(128, 128, NT)
        .transpose(0, 2, 1).reshape(128, NT * 128)
    )
    wblob_b[:, W_W2T : W_W2T + NT * 128] = w2tr.astype(bf)
    wblob_b[:, W_CWT : W_CWT + 128] = np.asarray(conv_w, dtype=f).T.astype(bf)
    wblob_b = np.ascontiguousarray(wblob_b)

    cb_arr = np.ascontiguousarray(np.asarray(conv_b, dtype=f).reshape(C, 1))
    mask_arr = np.ascontiguousarray(
        np.tile(np.eye(128, dtype=f), (1, NT)).astype(bf)
    )

    maps = []
    for bb in range(B):
        wblob_a = wblob_shared.copy()
        wblob_a[:, W_ALT] = altitude[bb].astype(bf)
        maps.append({"xpad": xq[bb], "wblob_a": np.ascontiguousarray(wblob_a),
                     "wblob_b": wblob_b, "cb": cb_arr, "mask": mask_arr})
    return maps


def kernel(x, altitude, W1, W2, conv_w, conv_b, ca_w1, ca_w2):
    global last_results
    in_maps = make_in_maps(x, altitude, W1, W2, conv_w, conv_b, ca_w1, ca_w2)
    nc = _get_module()
    trace = os.environ.get("KERNEL_TRACE", "0") == "1"
    last_results = run_bass_kernel_spmd(
        nc, in_maps, core_ids=list(range(B)), trace=trace
    )
    out = np.stack(
        [
            last_results.results[bb]["out"].astype(np.float32).reshape(C, H, W)
            for bb in range(B)
        ]
    )
    return out


# revision 16
# speedup vs baseline: 1.0106x; 1.0106x over previous
"""Trainium2 Bass kernel for nn_DA_conv: per-sample generated depthwise 3x3 conv
-> relu -> 1x1 pointwise conv (+bias) -> + x * channel_attention(altitude).

Data-parallel over batch: 8 samples -> 8 NeuronCores, weights replicated.

v8 design (trace-driven, from the 80.8us v5 and the 100us v7 experiment):
  * DVE 4x tensor_scalar mode requires an even row count per op (free size
    multiple of 256 elements): v7's 13-row slices silently halved DVE tap
    throughput (1070ns vs 683ns per TS). All chunks use dve_rows=12.
  * Rows PE 80 / DVE 48 (v5: 84/44). Measured rates: PE row 484ns, DVE row
    (9 TS @4x + 8 TT @2x tree) ~1.09us incl per-op overhead.
  * First ~6us are framework boot (engine TENSOR_LOADs); no kernel
    instruction can run earlier, so preamble DMA order only matters after
    that. wblob_a goes first on the sync queue (feat matmul is the head of
    the dependency chain); xp0 pieces ride the scalar queue in parallel.
  * No activation-table preload: Sigmoid and Relu live in different tables
    (v7's preload cost 2x1283ns on the Act queue ahead of the critical
    wblob_a trigger).
  * TAIL_LAG=3 + pss 3 bufs: the PE keeps a chunk of depthwise work queued
    ahead of tails so it never starves on the DVE tree (v7 starved, and the
    HAM clock thrashed between full and half speed in 3.4us epochs).
  * Chunk 3 is flipped (DVE rows on top, PE rows at the bottom) so the last
    tails consume PE-produced rows; its tail evacs alternate vector/scalar
    to drain the endgame faster.
  * Residual x*att rides the pointwise PSUM group as a diag(att) matmul.
    (gpsimd cannot run TensorScalarPtr/TensorTensor: Pool engine check fails
    at codegen; DVE STT runs at 1x so it is slower than the TS+TT tree.)
"""

import os
from collections import deque
from contextlib import ExitStack

import ml_dtypes
import numpy as np

import concourse.bass as bass
import concourse.mybir as mybir
import concourse.tile as tile
from concourse import bacc
from concourse.bass_utils import run_bass_kernel_spmd

AF = mybir.ActivationFunctionType
ALU = mybir.AluOpType
F32 = mybir.dt.float32
BF16 = mybir.dt.bfloat16

B, C, H, W = 8, 128, 128, 128
KK = 3
NT = KK * KK                 # 9 taps
HW = H * W
XOFF = 2                     # interior column offset in the padded layout
WP = W + 4                   # host-padded width (2 left, 2 right)
HP = H + 2                   # host-padded height (1 halo row each side)
R = 32                       # image rows per chunk
NCH = H // R                 # 4 chunks
PE_ROWS_L = [20, 20, 20, 20]  # per-chunk TensorE depthwise rows
FLIP_LAST = True             # chunk 3: DVE rows on top, PE rows at the bottom
TAPS = [(dy, dx) for dy in (-1, 0, 1) for dx in (-1, 0, 1)]  # t = (dy+1)*3+(dx+1)
N_WARM = 3                   # PE warm-up matmuls ahead of the feat matmul

# bf16 weight blob a: w1t | alt | ca1t | ca2t   (small, lands first)
W_W1T, W_ALT, W_CA1T, W_CA2T = 0, 128, 129, 145
WA_COLS = 145 + 128
# bf16 weight blob b: w2t (tap-major: col t*128+c) | cwt
W_W2T, W_CWT = 0, NT * 128
WB_COLS = NT * 128 + 128
WB_SPLIT = 5 * 128           # w2t columns for taps 0-4 (first diag half)
MASK_SPLIT = 5 * 128         # mask columns for the first diag-build half

last_results = None          # BassKernelResults of the most recent run


def _pe_blocks(pe_lo, pe_hi):
    blocks = []
    r = pe_lo
    while r < pe_hi:
        rr = min(4, pe_hi - r)
        blocks.append((r, r + rr))
        r += rr
    return blocks


def _emit(tc, nc, d):
    ctx = d["ctx"]
    singles = ctx.enter_context(tc.tile_pool(name="singles", bufs=1))
    xpool = ctx.enter_context(tc.tile_pool(name="xpool", bufs=3))
    spool = ctx.enter_context(tc.tile_pool(name="spool", bufs=3))
    tpool = ctx.enter_context(tc.tile_pool(name="tpool", bufs=3))
    opool = ctx.enter_context(tc.tile_pool(name="opool", bufs=6))
    pss_pool = ctx.enter_context(tc.tile_pool(name="psum_s", bufs=2, space="PSUM"))
    pso_pool = ctx.enter_context(tc.tile_pool(name="psum_o", bufs=3, space="PSUM"))

    # -- a few PE warm-up matmuls (HAM ramp), then the preamble DMAs on the
    # sync queue in v5's split order: the first wblob_b/mask halves unlock
    # ktab taps 0-4 and the first dg half while the rest streams in --
    warm = singles.tile([128, 512], BF16, name="warm", tag="warm")
    nc.gpsimd.memset(warm, 0.0)
    wps = pso_pool.tile([128, 512], F32, name="wps", tag="pso")
    for _ in range(N_WARM):
        nc.tensor.matmul(wps, lhsT=warm[:, 0:128], rhs=warm, start=True, stop=True)

    wblob_a = singles.tile([128, WA_COLS], BF16, name="wblob_a", tag="wblob_a")
    nc.sync.dma_start(out=wblob_a, in_=d["wblob_a"])
    wblob_b = singles.tile([128, WB_COLS], BF16, name="wblob_b", tag="wblob_b")
    nc.sync.dma_start(out=wblob_b[:, 0:WB_SPLIT], in_=d["wblob_b"][:, 0:WB_SPLIT])
    mask = singles.tile([128, NT * 128], BF16, name="mask", tag="mask")
    nc.sync.dma_start(out=mask[:, 0:MASK_SPLIT], in_=d["mask"][:, 0:MASK_SPLIT])

    x3 = d["xpad"].rearrange("c (h w) -> c h w", w=WP)
    xpf_d = d["xpad"]

    xps = []
    xp0 = xpool.tile([128, R + 2, WP], BF16, name="xp0", tag="xp")
    nc.sync.dma_start(out=xp0[:, 0:6, :], in_=x3[:, 0:6, :])
    nc.sync.dma_start(
        out=wblob_b[:, WB_SPLIT:WB_COLS], in_=d["wblob_b"][:, WB_SPLIT:WB_COLS]
    )
    nc.sync.dma_start(
        out=mask[:, MASK_SPLIT : NT * 128], in_=d["mask"][:, MASK_SPLIT : NT * 128]
    )
    nc.sync.dma_start(out=xp0[:, 6:18, :], in_=x3[:, 6:18, :])
    cb = singles.tile([128, 1], F32, name="cb", tag="cb")
    nc.sync.dma_start(out=cb, in_=d["cb"])
    nc.sync.dma_start(out=xp0[:, 18 : R + 2, :], in_=x3[:, 18 : R + 2, :])
    xps.append(xp0)
    _emit_prologue(tc, nc, d, singles, pss_pool, wblob_a, wblob_b, mask,
                   warm, wps)
    for ci in range(1, NCH):
        y0 = ci * R
        xp = xpool.tile([128, R + 2, WP], BF16, name=f"xp{ci}", tag="xp")
        nc.sync.dma_start(out=xp, in_=x3[:, y0 : y0 + R + 2, :])
        xps.append(xp)

    ktab = d["ktab"]
    dg_all = d["dg_all"]
    cwt = wblob_b[:, W_CWT : W_CWT + 128]
    out_d = d["out"]

    # PE-row-gated tails flush as soon as emitted; DVE-row-gated tails are
    # held until the NEXT chunk's PE blocks are in the queue, so the PE always
    # has independent depthwise work ahead of a tail that may wait on the DVE
    # tree -> scalar relu chain (v11 lost ~5us to those stalls).
    held_dve_tails = []

    for ci in range(NCH):
        y0 = ci * R
        flip = FLIP_LAST and ci == NCH - 1
        pe_rows = PE_ROWS_L[ci]
        dve_rows = R - pe_rows
        dve_lo = 0 if flip else pe_rows
        dve_hi = dve_rows if flip else R
        pe_lo = dve_rows if flip else 0
        pe_hi = R if flip else pe_rows
        xp = xps[ci]
        srelu = spool.tile([128, R * W], BF16, name=f"sr{ci}", tag="sr")

        # -- DVE depthwise rows [dve_lo, dve_hi): 9 TS products into slots of
        # one 4D tile, then a contiguous-halves add tree (4 TTs, big APs) --
        P = tpool.tile([128, 8, dve_rows, W], BF16, name=f"P{ci}", tag="P")
        t8 = tpool.tile([128, dve_rows, W], BF16, name=f"t8{ci}", tag="t8")
        for ti, (dy, dx) in enumerate(TAPS):
            src = xp[:, 1 + dve_lo + dy : 1 + dve_hi + dy,
                     XOFF + dx : XOFF + dx + W]
            out = t8 if ti == 8 else P[:, ti]
            nc.vector.tensor_scalar_mul(out=out, in0=src,
                                        scalar1=ktab[:, ti : ti + 1])
        nc.vector.tensor_tensor(out=P[:, 0:4], in0=P[:, 0:4], in1=P[:, 4:8],
                                op=ALU.add)
        nc.vector.tensor_tensor(out=P[:, 0:2], in0=P[:, 0:2], in1=P[:, 2:4],
                                op=ALU.add)
        nc.vector.tensor_tensor(out=P[:, 0], in0=P[:, 0], in1=P[:, 1],
                                op=ALU.add)
        nc.vector.tensor_tensor(out=P[:, 0], in0=P[:, 0], in1=t8, op=ALU.add)
        nc.scalar.activation(srelu[:, dve_lo * W : dve_hi * W], P[:, 0],
                             AF.Relu)
        if ci == 0:
            d["emit_attention"]()
            attd = d["attd"]

        # -- PE depthwise: rows [pe_lo, pe_hi) in <=4-row PSUM blocks --
        for rs, re in _pe_blocks(pe_lo, pe_hi):
            rows = re - rs
            pss = pss_pool.tile([128, rows * W], F32, name=f"pss{ci}_{rs}",
                                tag="pss")
            for ti, (dy, dx) in enumerate(TAPS):
                rhs = xp[:, 1 + rs + dy : 1 + re + dy, XOFF + dx : XOFF + dx + W]
                nc.tensor.matmul(
                    pss, lhsT=dg_all[:, ti * 128 : (ti + 1) * 128], rhs=rhs,
                    start=(ti == 0), stop=(ti == NT - 1),
                )
            nc.scalar.activation(srelu[:, rs * W : re * W], pss, AF.Relu)

        # previous chunk's DVE-gated tails now have a chunk of PE blocks ahead
        for t in held_dve_tails:
            t()
        held_dve_tails = []

        # -- tails: 8-row units (2 PSUM banks of pointwise+residual, 1 store) --
        last = ci == NCH - 1
        for ti_, tr in enumerate(range(0, R, 8)):
            dve_gated = (tr + 8 > dve_lo) and (tr < dve_hi)
            evac_vec = last and (ti_ % 2 == 0)
            t = _make_tail(nc, pso_pool, opool, xp, srelu, cwt, attd,
                           cb, out_d, ci, tr, y0, evac_vec)
            if dve_gated and not last:
                held_dve_tails.append(t)
            else:
                t()
    for t in held_dve_tails:
        t()


def _emit_prologue(tc, nc, d, singles, pss_pool, wblob_a, wblob_b, mask,
                   warm, wps):
    def bridge(n):
        # fill PE-idle dependency stalls so the HAM clock never re-throttles
        for _ in range(n):
            nc.tensor.matmul(wps, lhsT=warm[:, 0:128], rhs=warm,
                             start=True, stop=True)

    alt = wblob_a[:, W_ALT : W_ALT + 1]
    w1t = wblob_a[:, W_W1T : W_W1T + 128]
    ca1t = wblob_a[:, W_CA1T : W_CA1T + 16]
    ca2t = wblob_a[0:16, W_CA2T : W_CA2T + 128]
    w2t = wblob_b[:, W_W2T : W_W2T + NT * 128]

    def leaky(name, psum_src, parts, dt=F32):
        tmp = singles.tile([parts, 1], F32, name=f"{name}_t", tag=f"{name}_t")
        nc.vector.tensor_scalar_mul(out=tmp, in0=psum_src, scalar1=0.1)
        res = singles.tile([parts, 1], dt, name=name, tag=name)
        nc.vector.tensor_tensor(out=res, in0=tmp, in1=psum_src, op=ALU.max)
        return res

    # ---- kernel-generator MLP (all bf16 matmuls) ----
    feat_ps = pss_pool.tile([128, 1], F32, name="feat_ps", tag="pss")
    nc.tensor.matmul(feat_ps, lhsT=w1t, rhs=alt, start=True, stop=True)
    bridge(2)
    feat = leaky("feat", feat_ps, 128, dt=BF16)

    ktab_ps = pss_pool.tile([128, NT], F32, name="ktab_ps", tag="pss")
    for t in range(5):
        nc.tensor.matmul(
            ktab_ps[:, t : t + 1], lhsT=w2t[:, t * 128 : (t + 1) * 128],
            rhs=feat, start=True, stop=True
        )
    bridge(1)
    for t in range(5, NT):
        nc.tensor.matmul(
            ktab_ps[:, t : t + 1], lhsT=w2t[:, t * 128 : (t + 1) * 128],
            rhs=feat, start=True, stop=True
        )
    bridge(2)
    # SBUF copy (DVE tap scalar source) on the vector engine, ahead of the
    # chunk-0 taps in its queue; the dg build reads ktab_ps straight from PSUM.
    ktab = singles.tile([128, NT], F32, name="ktab", tag="ktab")
    nc.vector.tensor_scalar_add(out=ktab, in0=ktab_ps, scalar1=0.0)

    # ---- diag weights straight from PSUM: dg[:, t*128+j] = I[p,j]*ktab[p,t]
    dg_all = singles.tile([128, NT * 128], BF16, name="dg_all", tag="dg_all")
    ktab_b = ktab_ps.unsqueeze(2).broadcast_to([128, NT, 128])
    mask3 = mask.rearrange("p (t c) -> p t c", t=NT)
    dg3 = dg_all.rearrange("p (t c) -> p t c", t=NT)
    nc.vector.tensor_tensor(
        out=dg3[:, 0:5, :], in0=mask3[:, 0:5, :], in1=ktab_b[:, 0:5, :],
        op=ALU.mult,
    )
    nc.vector.tensor_tensor(
        out=dg3[:, 5:NT, :], in0=mask3[:, 5:NT, :], in1=ktab_b[:, 5:NT, :],
        op=ALU.mult,
    )

    # keep the PE busy across the preamble->body handoff: an idle epoch here
    # makes the HAM down-throttle right as the first depthwise blocks issue
    # (the dg chain completes ~15us: boot+barrier+DMA flow are fixed costs)
    bridge(12)
    d["ktab"] = ktab
    d["dg_all"] = dg_all

    def emit_attention():
        # deferred until after chunk 0's tap emission: the Sigmoid act-table
        # load (~1.3us on the scalar engine) and the attd build must not gate
        # the first DVE taps; attd is first needed by the chunk-0 tails.
        a1_ps = pss_pool.tile([16, 1], F32, name="a1_ps", tag="pss")
        nc.tensor.matmul(a1_ps, lhsT=ca1t, rhs=alt, start=True, stop=True)
        a1 = leaky("a1", a1_ps, 16, dt=BF16)
        att_ps = pss_pool.tile([128, 1], F32, name="att_ps", tag="pss")
        nc.tensor.matmul(att_ps, lhsT=ca2t, rhs=a1, start=True, stop=True)
        attv = singles.tile([128, 1], F32, name="attv", tag="attv")
        nc.scalar.activation(attv, att_ps, AF.Sigmoid)
        attd = singles.tile([128, 128], BF16, name="attd", tag="attd")
        nc.vector.tensor_tensor(
            out=attd, in0=mask[:, 0:128], in1=attv.broadcast_to([128, 128]),
            op=ALU.mult,
        )
        d["attd"] = attd
    d["emit_attention"] = emit_attention


def _make_tail(nc, pso_pool, opool, xp, srelu, cwt, attd, cb, out_d, ci, tr,
               y0, evac_vec=False):
    """Two 4-row pointwise+residual groups into one 2-bank PSUM tile, single
    biased bf16 evac and store for chunk-relative rows [tr, tr+8)."""

    def tail():
        osb = opool.tile([128, 8 * W], BF16, name=f"ob{ci}_{tr}", tag="ob")
        pso = pso_pool.tile([128, 8 * W], F32, name=f"pso{ci}_{tr}", tag="pso")
        for h, r0 in enumerate((tr, tr + 4)):
            half = pso[:, h * 4 * W : (h + 1) * 4 * W]
            nc.tensor.matmul(half, lhsT=cwt, rhs=srelu[:, r0 * W : (r0 + 4) * W],
                             start=True, stop=False)
            nc.tensor.matmul(
                half, lhsT=attd, rhs=xp[:, 1 + r0 : 1 + r0 + 4, XOFF : XOFF + W],
                start=False, stop=True,
            )
        if evac_vec:
            nc.vector.tensor_scalar_add(out=osb, in0=pso, scalar1=cb)
        else:
            nc.scalar.activation(osb, pso, AF.Identity, bias=cb)
        nc.gpsimd.dma_start(
            out=out_d[:, (y0 + tr) * W : (y0 + tr + 8) * W], in_=osb
        )

    return tail


def build_module():
    nc = bacc.Bacc(
        "TRN2",
        target_bir_lowering=False,
        debug=False,
        enable_asserts=False,
        num_devices=B,
    )
    d = {
        "xpad": nc.dram_tensor("xpad", [C, HP * WP], BF16, kind="ExternalInput").ap(),
        "wblob_a": nc.dram_tensor("wblob_a", [128, WA_COLS], BF16, kind="ExternalInput").ap(),
        "wblob_b": nc.dram_tensor("wblob_b", [128, WB_COLS], BF16, kind="ExternalInput").ap(),
        "cb": nc.dram_tensor("cb", [C, 1], F32, kind="ExternalInput").ap(),
        "mask": nc.dram_tensor("mask", [128, NT * 128], BF16, kind="ExternalInput").ap(),
        "out": nc.dram_tensor("out", [C, HW], BF16, kind="ExternalOutput").ap(),
    }
    with tile.TileContext(nc) as tc:
        with ExitStack() as ctx:
            d["ctx"] = ctx
            _emit(tc, nc, d)
    nc.finalize()
    return nc


_module_cache = None


def _get_module():
    global _module_cache
    if _module_cache is None:
        _module_cache = build_module()
    return _module_cache


def make_in_maps(x, altitude, W1, W2, conv_w, conv_b, ca_w1, ca_w2):
    f = np.float32
    bf = ml_dtypes.bfloat16
    x = np.asarray(x, dtype=f)
    altitude = np.asarray(altitude, dtype=f)
    xpad = np.zeros((B, C, HP, WP), dtype=f)
    xpad[:, :, 1 : H + 1, XOFF : XOFF + W] = x
    xq = np.ascontiguousarray(xpad.astype(bf).reshape(B, C, HP * WP))

    wblob_shared = np.zeros((128, WA_COLS), dtype=bf)
    wblob_shared[:, W_W1T : W_W1T + 128] = np.asarray(W1, dtype=f).T.astype(bf)
    wblob_shared[:, W_CA1T : W_CA1T + 16] = np.asarray(ca_w1, dtype=f).T.astype(bf)
    wblob_shared[0:16, W_CA2T : W_CA2T + 128] = np.asarray(
        ca_w2, dtype=f
    ).T.astype(bf)
    wblob_b = np.zeros((128, WB_COLS), dtype=bf)
    w2tr = (
        np.asarray(W2, dtype=f).T.reshape(128, 128, NT)
        .transpose(0, 2, 1).reshape(128, NT * 128)
    )
    wblob_b[:, W_W2T : W_W2T + NT * 128] = w2tr.astype(bf)
    wblob_b[:, W_CWT : W_CWT + 128] = np.asarray(conv_w, dtype=f).T.astype(bf)
    wblob_b = np.ascontiguousarray(wblob_b)

    cb_arr = np.ascontiguousarray(np.asarray(conv_b, dtype=f).reshape(C, 1))
    mask_arr = np.ascontiguousarray(
        np.tile(np.eye(128, dtype=f), (1, NT)).astype(bf)
    )

    maps = []
    for bb in range(B):
        wblob_a = wblob_shared.copy()
        wblob_a[:, W_ALT] = altitude[bb].astype(bf)
        maps.append({"xpad": xq[bb], "wblob_a": np.ascontiguousarray(wblob_a),
                     "wblob_b": wblob_b, "cb": cb_arr, "mask": mask_arr})
    return maps


def kernel(x, altitude, W1, W2, conv_w, conv_b, ca_w1, ca_w2):
    global last_results
    in_maps = make_in_maps(x, altitude, W1, W2, conv_w, conv_b, ca_w1, ca_w2)
    nc = _get_module()
    trace = os.environ.get("KERNEL_TRACE", "0") == "1"
    last_results = run_bass_kernel_spmd(
        nc, in_maps, core_ids=list(range(B)), trace=trace
    )
    out = np.stack(
        [
            last_results.results[bb]["out"].astype(np.float32).reshape(C, H, W)
            for bb in range(B)
        ]
    )
    return out


# revision 17
# speedup vs baseline: 1.0501x; 1.0391x over previous
"""Trainium2 Bass kernel for nn_DA_conv: per-sample generated depthwise 3x3 conv
-> relu -> 1x1 pointwise conv (+bias) -> + x * channel_attention(altitude).

Data-parallel over batch: 8 samples -> 8 NeuronCores, weights replicated.

v8 design (trace-driven, from the 80.8us v5 and the 100us v7 experiment):
  * DVE 4x tensor_scalar mode requires an even row count per op (free size
    multiple of 256 elements): v7's 13-row slices silently halved DVE tap
    throughput (1070ns vs 683ns per TS). All chunks use dve_rows=12.
  * Rows PE 80 / DVE 48 (v5: 84/44). Measured rates: PE row 484ns, DVE row
    (9 TS @4x + 8 TT @2x tree) ~1.09us incl per-op overhead.
  * First ~6us are framework boot (engine TENSOR_LOADs); no kernel
    instruction can run earlier, so preamble DMA order only matters after
    that. wblob_a goes first on the sync queue (feat matmul is the head of
    the dependency chain); xp0 pieces ride the scalar queue in parallel.
  * No activation-table preload: Sigmoid and Relu live in different tables
    (v7's preload cost 2x1283ns on the Act queue ahead of the critical
    wblob_a trigger).
  * TAIL_LAG=3 + pss 3 bufs: the PE keeps a chunk of depthwise work queued
    ahead of tails so it never starves on the DVE tree (v7 starved, and the
    HAM clock thrashed between full and half speed in 3.4us epochs).
  * Chunk 3 is flipped (DVE rows on top, PE rows at the bottom) so the last
    tails consume PE-produced rows; its tail evacs alternate vector/scalar
    to drain the endgame faster.
  * Residual x*att rides the pointwise PSUM group as a diag(att) matmul.
    (gpsimd cannot run TensorScalarPtr/TensorTensor: Pool engine check fails
    at codegen; DVE STT runs at 1x so it is slower than the TS+TT tree.)
"""

import os
from collections import deque
from contextlib import ExitStack

import ml_dtypes
import numpy as np

import concourse.bass as bass
import concourse.mybir as mybir
import concourse.tile as tile
from concourse import bacc
from concourse.bass_utils import run_bass_kernel_spmd

AF = mybir.ActivationFunctionType
ALU = mybir.AluOpType
F32 = mybir.dt.float32
BF16 = mybir.dt.bfloat16

B, C, H, W = 8, 128, 128, 128
KK = 3
NT = KK * KK                 # 9 taps
HW = H * W
XOFF = 2                     # interior column offset in the padded layout
WP = W + 4                   # host-padded width (2 left, 2 right)
HP = H + 2                   # host-padded height (1 halo row each side)
R = 32                       # image rows per chunk
NCH = H // R                 # 4 chunks
PE_ROWS_L = [20, 20, 20, 20]  # per-chunk TensorE depthwise rows
FLIP_LAST = True             # chunk 3: DVE rows on top, PE rows at the bottom
TAPS = [(dy, dx) for dy in (-1, 0, 1) for dx in (-1, 0, 1)]  # t = (dy+1)*3+(dx+1)
N_WARM = 3                   # PE warm-up matmuls ahead of the feat matmul

# bf16 weight blob a: w1t | alt | ca1t | ca2t   (small, lands first)
W_W1T, W_ALT, W_CA1T, W_CA2T = 0, 128, 129, 145
WA_COLS = 145 + 128
# bf16 weight blob b: w2t (tap-major: col t*128+c) | cwt
W_W2T, W_CWT = 0, NT * 128
WB_COLS = NT * 128 + 128
WB_SPLIT = 5 * 128           # w2t columns for taps 0-4 (first diag half)
MASK_SPLIT = 5 * 128         # mask columns for the first diag-build half

last_results = None          # BassKernelResults of the most recent run


def _pe_blocks(pe_lo, pe_hi):
    blocks = []
    r = pe_lo
    while r < pe_hi:
        rr = min(4, pe_hi - r)
        blocks.append((r, r + rr))
        r += rr
    return blocks


def _emit(tc, nc, d):
    ctx = d["ctx"]
    singles = ctx.enter_context(tc.tile_pool(name="singles", bufs=1))
    xpool = ctx.enter_context(tc.tile_pool(name="xpool", bufs=3))
    spool = ctx.enter_context(tc.tile_pool(name="spool", bufs=3))
    tpool = ctx.enter_context(tc.tile_pool(name="tpool", bufs=3))
    opool = ctx.enter_context(tc.tile_pool(name="opool", bufs=4))
    pss_pool = ctx.enter_context(tc.tile_pool(name="psum_s", bufs=2, space="PSUM"))
    pso_pool = ctx.enter_context(tc.tile_pool(name="psum_o", bufs=3, space="PSUM"))

    # -- a few PE warm-up matmuls (HAM ramp), then the preamble DMAs on the
    # sync queue in v5's split order: the first wblob_b/mask halves unlock
    # ktab taps 0-4 and the first dg half while the rest streams in --
    warm = singles.tile([128, 512], BF16, name="warm", tag="warm")
    nc.gpsimd.memset(warm, 0.0)
    wps = pso_pool.tile([128, 512], F32, name="wps", tag="pso")
    for _ in range(N_WARM):
        nc.tensor.matmul(wps, lhsT=warm[:, 0:128], rhs=warm, start=True, stop=True)

    wblob_a = singles.tile([128, WA_COLS], BF16, name="wblob_a", tag="wblob_a")
    nc.sync.dma_start(out=wblob_a, in_=d["wblob_a"])
    wblob_b = singles.tile([128, WB_COLS], BF16, name="wblob_b", tag="wblob_b")
    nc.sync.dma_start(out=wblob_b[:, 0:WB_SPLIT], in_=d["wblob_b"][:, 0:WB_SPLIT])
    mask = singles.tile([128, NT * 128], BF16, name="mask", tag="mask")
    nc.sync.dma_start(out=mask[:, 0:MASK_SPLIT], in_=d["mask"][:, 0:MASK_SPLIT])

    x3 = d["xpad"].rearrange("c (h w) -> c h w", w=WP)
    xpf_d = d["xpad"]

    xps = []
    xp0 = xpool.tile([128, R + 2, WP], BF16, name="xp0", tag="xp")
    nc.sync.dma_start(out=xp0[:, 0:6, :], in_=x3[:, 0:6, :])
    nc.sync.dma_start(
        out=wblob_b[:, WB_SPLIT:WB_COLS], in_=d["wblob_b"][:, WB_SPLIT:WB_COLS]
    )
    nc.sync.dma_start(
        out=mask[:, MASK_SPLIT : NT * 128], in_=d["mask"][:, MASK_SPLIT : NT * 128]
    )
    nc.sync.dma_start(out=xp0[:, 6:18, :], in_=x3[:, 6:18, :])
    cb = singles.tile([128, 1], F32, name="cb", tag="cb")
    nc.sync.dma_start(out=cb, in_=d["cb"])
    nc.sync.dma_start(out=xp0[:, 18 : R + 2, :], in_=x3[:, 18 : R + 2, :])
    xps.append(xp0)
    _emit_prologue(tc, nc, d, singles, pss_pool, wblob_a, wblob_b, mask,
                   warm, wps)
    for ci in range(1, NCH):
        y0 = ci * R
        xp = xpool.tile([128, R + 2, WP], BF16, name=f"xp{ci}", tag="xp")
        nc.sync.dma_start(out=xp, in_=x3[:, y0 : y0 + R + 2, :])
        xps.append(xp)

    ktab = d["ktab"]
    dg_all = d["dg_all"]
    cwt = wblob_b[:, W_CWT : W_CWT + 128]
    out_d = d["out"]

    # PE-row-gated tails flush as soon as emitted; DVE-row-gated tails are
    # held until the NEXT chunk's PE blocks are in the queue, so the PE always
    # has independent depthwise work ahead of a tail that may wait on the DVE
    # tree -> scalar relu chain (v11 lost ~5us to those stalls).
    held_dve_tails = []

    for ci in range(NCH):
        y0 = ci * R
        flip = FLIP_LAST and ci == NCH - 1
        pe_rows = PE_ROWS_L[ci]
        dve_rows = R - pe_rows
        dve_lo = 0 if flip else pe_rows
        dve_hi = dve_rows if flip else R
        pe_lo = dve_rows if flip else 0
        pe_hi = R if flip else pe_rows
        xp = xps[ci]
        srelu = spool.tile([128, R * W], BF16, name=f"sr{ci}", tag="sr")

        # -- DVE depthwise rows [dve_lo, dve_hi): 9 TS products into slots of
        # one 4D tile, then a contiguous-halves add tree (4 TTs, big APs) --
        P = tpool.tile([128, 8, dve_rows, W], BF16, name=f"P{ci}", tag="P")
        t8 = tpool.tile([128, dve_rows, W], BF16, name=f"t8{ci}", tag="t8")
        for ti, (dy, dx) in enumerate(TAPS):
            src = xp[:, 1 + dve_lo + dy : 1 + dve_hi + dy,
                     XOFF + dx : XOFF + dx + W]
            out = t8 if ti == 8 else P[:, ti]
            nc.vector.tensor_scalar_mul(out=out, in0=src,
                                        scalar1=ktab[:, ti : ti + 1])
        nc.vector.tensor_tensor(out=P[:, 0:4], in0=P[:, 0:4], in1=P[:, 4:8],
                                op=ALU.add)
        nc.vector.tensor_tensor(out=P[:, 0:2], in0=P[:, 0:2], in1=P[:, 2:4],
                                op=ALU.add)
        nc.vector.tensor_tensor(out=P[:, 0], in0=P[:, 0], in1=P[:, 1],
                                op=ALU.add)
        nc.vector.tensor_tensor(out=P[:, 0], in0=P[:, 0], in1=t8, op=ALU.add)
        nc.scalar.activation(srelu[:, dve_lo * W : dve_hi * W], P[:, 0],
                             AF.Relu)
        if ci == 0:
            d["emit_attention"]()
            attd = d["attd"]

        # -- PE depthwise: rows [pe_lo, pe_hi) in <=4-row PSUM blocks, with
        # the previous chunk's DVE-gated tails interleaved between blocks so
        # their evacs/stores don't burst on the scalar engine and PSUM --
        for bi, (rs, re) in enumerate(_pe_blocks(pe_lo, pe_hi)):
            rows = re - rs
            pss = pss_pool.tile([128, rows * W], F32, name=f"pss{ci}_{rs}",
                                tag="pss")
            for ti, (dy, dx) in enumerate(TAPS):
                rhs = xp[:, 1 + rs + dy : 1 + re + dy, XOFF + dx : XOFF + dx + W]
                nc.tensor.matmul(
                    pss, lhsT=dg_all[:, ti * 128 : (ti + 1) * 128], rhs=rhs,
                    start=(ti == 0), stop=(ti == NT - 1),
                )
            nc.scalar.activation(srelu[:, rs * W : re * W], pss, AF.Relu)
            if bi in (1, 3) and held_dve_tails:
                held_dve_tails.pop(0)()
        for t in held_dve_tails:
            t()
        held_dve_tails = []

        # -- tails: 8-row pointwise+residual units; pairs share one 16-row
        # osb tile and the pair's second tail fires a single 16-row store --
        last = ci == NCH - 1
        osb_pair = {}
        for ti_, tr in enumerate(range(0, R, 8)):
            dve_gated = (tr + 8 > dve_lo) and (tr < dve_hi)
            evac_vec = last and (ti_ % 2 == 0)
            t = _make_tail(nc, pso_pool, opool, xp, srelu, cwt, attd,
                           cb, out_d, ci, tr, y0, evac_vec, osb_pair)
            if dve_gated and not last:
                held_dve_tails.append(t)
            else:
                t()
    for t in held_dve_tails:
        t()


def _emit_prologue(tc, nc, d, singles, pss_pool, wblob_a, wblob_b, mask,
                   warm, wps):
    def bridge(n):
        # fill PE-idle dependency stalls so the HAM clock never re-throttles
        for _ in range(n):
            nc.tensor.matmul(wps, lhsT=warm[:, 0:128], rhs=warm,
                             start=True, stop=True)

    alt = wblob_a[:, W_ALT : W_ALT + 1]
    w1t = wblob_a[:, W_W1T : W_W1T + 128]
    ca1t = wblob_a[:, W_CA1T : W_CA1T + 16]
    ca2t = wblob_a[0:16, W_CA2T : W_CA2T + 128]
    w2t = wblob_b[:, W_W2T : W_W2T + NT * 128]

    def leaky(name, psum_src, parts, dt=F32):
        tmp = singles.tile([parts, 1], F32, name=f"{name}_t", tag=f"{name}_t")
        nc.vector.tensor_scalar_mul(out=tmp, in0=psum_src, scalar1=0.1)
        res = singles.tile([parts, 1], dt, name=name, tag=name)
        nc.vector.tensor_tensor(out=res, in0=tmp, in1=psum_src, op=ALU.max)
        return res

    # ---- kernel-generator MLP (all bf16 matmuls) ----
    feat_ps = pss_pool.tile([128, 1], F32, name="feat_ps", tag="pss")
    nc.tensor.matmul(feat_ps, lhsT=w1t, rhs=alt, start=True, stop=True)
    bridge(2)
    feat = leaky("feat", feat_ps, 128, dt=BF16)

    ktab_ps = pss_pool.tile([128, NT], F32, name="ktab_ps", tag="pss")
    for t in range(5):
        nc.tensor.matmul(
            ktab_ps[:, t : t + 1], lhsT=w2t[:, t * 128 : (t + 1) * 128],
            rhs=feat, start=True, stop=True
        )
    bridge(1)
    for t in range(5, NT):
        nc.tensor.matmul(
            ktab_ps[:, t : t + 1], lhsT=w2t[:, t * 128 : (t + 1) * 128],
            rhs=feat, start=True, stop=True
        )
    bridge(2)
    # SBUF copy (DVE tap scalar source) on the vector engine, ahead of the
    # chunk-0 taps in its queue; the dg build reads ktab_ps straight from PSUM.
    ktab = singles.tile([128, NT], F32, name="ktab", tag="ktab")
    nc.vector.tensor_scalar_add(out=ktab, in0=ktab_ps, scalar1=0.0)

    # ---- diag weights straight from PSUM: dg[:, t*128+j] = I[p,j]*ktab[p,t]
    dg_all = singles.tile([128, NT * 128], BF16, name="dg_all", tag="dg_all")
    ktab_b = ktab_ps.unsqueeze(2).broadcast_to([128, NT, 128])
    mask3 = mask.rearrange("p (t c) -> p t c", t=NT)
    dg3 = dg_all.rearrange("p (t c) -> p t c", t=NT)
    nc.vector.tensor_tensor(
        out=dg3[:, 0:5, :], in0=mask3[:, 0:5, :], in1=ktab_b[:, 0:5, :],
        op=ALU.mult,
    )
    nc.vector.tensor_tensor(
        out=dg3[:, 5:NT, :], in0=mask3[:, 5:NT, :], in1=ktab_b[:, 5:NT, :],
        op=ALU.mult,
    )

    # keep the PE busy across the preamble->body handoff: an idle epoch here
    # makes the HAM down-throttle right as the first depthwise blocks issue
    # (the dg chain completes ~15us: boot+barrier+DMA flow are fixed costs)
    bridge(12)
    d["ktab"] = ktab
    d["dg_all"] = dg_all

    def emit_attention():
        # deferred until after chunk 0's tap emission: the Sigmoid act-table
        # load (~1.3us on the scalar engine) and the attd build must not gate
        # the first DVE taps; attd is first needed by the chunk-0 tails.
        a1_ps = pss_pool.tile([16, 1], F32, name="a1_ps", tag="pss")
        nc.tensor.matmul(a1_ps, lhsT=ca1t, rhs=alt, start=True, stop=True)
        a1 = leaky("a1", a1_ps, 16, dt=BF16)
        att_ps = pss_pool.tile([128, 1], F32, name="att_ps", tag="pss")
        nc.tensor.matmul(att_ps, lhsT=ca2t, rhs=a1, start=True, stop=True)
        attv = singles.tile([128, 1], F32, name="attv", tag="attv")
        nc.scalar.activation(attv, att_ps, AF.Sigmoid)
        attd = singles.tile([128, 128], BF16, name="attd", tag="attd")
        nc.vector.tensor_tensor(
            out=attd, in0=mask[:, 0:128], in1=attv.broadcast_to([128, 128]),
            op=ALU.mult,
        )
        d["attd"] = attd
    d["emit_attention"] = emit_attention


def _make_tail(nc, pso_pool, opool, xp, srelu, cwt, attd, cb, out_d, ci, tr,
               y0, evac_vec=False, osb_pair=None):
    """Two 4-row pointwise+residual groups into one 2-bank PSUM tile, biased
    bf16 evac into half of a shared 16-row osb; the pair's second tail
    stores all 16 rows in one DMA (bigger descriptors, half the triggers)."""

    def tail():
        pk = tr // 16
        if pk in osb_pair:
            osb16 = osb_pair.pop(pk)
            second = True
        else:
            osb16 = opool.tile([128, 16 * W], BF16, name=f"ob{ci}_{pk}",
                               tag="ob")
            osb_pair[pk] = osb16
            second = False
        osb = osb16[:, (tr % 16) * W : (tr % 16 + 8) * W]
        pso = pso_pool.tile([128, 8 * W], F32, name=f"pso{ci}_{tr}", tag="pso")
        for h, r0 in enumerate((tr, tr + 4)):
            half = pso[:, h * 4 * W : (h + 1) * 4 * W]
            nc.tensor.matmul(half, lhsT=cwt, rhs=srelu[:, r0 * W : (r0 + 4) * W],
                             start=True, stop=False)
            nc.tensor.matmul(
                half, lhsT=attd, rhs=xp[:, 1 + r0 : 1 + r0 + 4, XOFF : XOFF + W],
                start=False, stop=True,
            )
        if evac_vec:
            nc.vector.tensor_scalar_add(out=osb, in0=pso, scalar1=cb)
        else:
            nc.scalar.activation(osb, pso, AF.Identity, bias=cb)
        if second:
            nc.gpsimd.dma_start(
                out=out_d[:, (y0 + pk * 16) * W : (y0 + pk * 16 + 16) * W],
                in_=osb16,
            )

    return tail


def build_module():
    nc = bacc.Bacc(
        "TRN2",
        target_bir_lowering=False,
        debug=False,
        enable_asserts=False,
        num_devices=B,
    )
    d = {
        "xpad": nc.dram_tensor("xpad", [C, HP * WP], BF16, kind="ExternalInput").ap(),
        "wblob_a": nc.dram_tensor("wblob_a", [128, WA_COLS], BF16, kind="ExternalInput").ap(),
        "wblob_b": nc.dram_tensor("wblob_b", [128, WB_COLS], BF16, kind="ExternalInput").ap(),
        "cb": nc.dram_tensor("cb", [C, 1], F32, kind="ExternalInput").ap(),
        "mask": nc.dram_tensor("mask", [128, NT * 128], BF16, kind="ExternalInput").ap(),
        "out": nc.dram_tensor("out", [C, HW], BF16, kind="ExternalOutput").ap(),
    }
    with tile.TileContext(nc) as tc:
        with ExitStack() as ctx:
            d["ctx"] = ctx
            _emit(tc, nc, d)
    nc.finalize()
    return nc


_module_cache = None


def _get_module():
    global _module_cache
    if _module_cache is None:
        _module_cache = build_module()
    return _module_cache


def make_in_maps(x, altitude, W1, W2, conv_w, conv_b, ca_w1, ca_w2):
    f = np.float32
    bf = ml_dtypes.bfloat16
    x = np.asarray(x, dtype=f)
    altitude = np.asarray(altitude, dtype=f)
    xpad = np.zeros((B, C, HP, WP), dtype=f)
    xpad[:, :, 1 : H + 1, XOFF : XOFF + W] = x
    xq = np.ascontiguousarray(xpad.astype(bf).reshape(B, C, HP * WP))

    wblob_shared = np.zeros((128, WA_COLS), dtype=bf)
    wblob_shared[:, W_W1T : W_W1T + 128] = np.asarray(W1, dtype=f).T.astype(bf)
    wblob_shared[:, W_CA1T : W_CA1T + 16] = np.asarray(ca_w1, dtype=f).T.astype(bf)
    wblob_shared[0:16, W_CA2T : W_CA2T + 128] = np.asarray(
        ca_w2, dtype=f
    ).T.astype(bf)
    wblob_b = np.zeros((128, WB_COLS), dtype=bf)
    w2tr = (
        np.asarray(W2, dtype=f).T.reshape(128, 128, NT)
        .transpose(0, 2, 1).reshape(128, NT * 128)
    )
    wblob_b[:, W_W2T : W_W2T + NT * 128] = w2tr.astype(bf)
    wblob_b[:, W_CWT : W_CWT + 128] = np.asarray(conv_w, dtype=f).T.astype(bf)
    wblob_b = np.ascontiguousarray(wblob_b)

    cb_arr = np.ascontiguousarray(np.asarray(conv_b, dtype=f).reshape(C, 1))
    mask_arr = np.ascontiguousarray(
        np.tile(np.eye(128, dtype=f), (1, NT)).astype(bf)
    )

    maps = []
    for bb in range(B):
        wblob_a = wblob_shared.copy()
        wblob_a[:, W_ALT] = altitude[bb].astype(bf)
        maps.append({"xpad": xq[bb], "wblob_a": np.ascontiguousarray(wblob_a),
                     "wblob_b": wblob_b, "cb": cb_arr, "mask": mask_arr})
    return maps


def kernel(x, altitude, W1, W2, conv_w, conv_b, ca_w1, ca_w2):
    global last_results
    in_maps = make_in_maps(x, altitude, W1, W2, conv_w, conv_b, ca_w1, ca_w2)
    nc = _get_module()
    trace = os.environ.get("KERNEL_TRACE", "0") == "1"
    last_results = run_bass_kernel_spmd(
        nc, in_maps, core_ids=list(range(B)), trace=trace
    )
    out = np.stack(
        [
            last_results.results[bb]["out"].astype(np.float32).reshape(C, H, W)
            for bb in range(B)
        ]
    )
    return out


# revision 18
# speedup vs baseline: 1.0558x; 1.0055x over previous
"""Trainium2 Bass kernel for nn_DA_conv: per-sample generated depthwise 3x3 conv
-> relu -> 1x1 pointwise conv (+bias) -> + x * channel_attention(altitude).

Data-parallel over batch: 8 samples -> 8 NeuronCores, weights replicated.

v18 design (trace-driven; v5 baseline 80.8us -> 77.7us):
  * Rows PE 80 / DVE 48. Measured rates: PE depthwise row (9 diag bf16
    matmuls, N=512 issue interval 215ns) 484ns; DVE row ~1.04us.
  * DVE 4x tensor_scalar mode requires an even row count per op (free size
    multiple of 256 elements); odd-element column offsets are fine (the old
    xb1 shifted-copy DMA was unnecessary - all taps read xp directly).
  * DVE tap products write slots of one 4D tile [128,8,rows,W]; the add tree
    is 4 contiguous-halves tensor_tensors with big APs (fewer instructions,
    ~0.6us/chunk less DVE time than a pairwise tree over 9 tiles).
  * Tails (pointwise cwt + diag(att) residual into 2 PSUM banks, biased bf16
    evac) are split by dependency: PE-row tails flush immediately; DVE-row-
    gated tails are held and interleaved between the NEXT chunk's depthwise
    blocks, so the PE always has independent work queued ahead of a tail
    that may wait on the DVE tree -> relu chain.
  * Tail pairs share a 16-row osb tile; one 16-row store per pair (bigger
    DMA descriptors, half the SWDGE triggers, less osb recycle pressure -
    store completion gates osb reuse and stalled the PE in v15/earlier).
  * Chunk 3 is flipped (DVE rows on top) so the final tails consume
    PE-produced rows; its evacs alternate vector/scalar to drain faster.
  * HAM clock: k=4/8 at boot, lifts after ~1-2 epochs (3.4us) of sustained
    PE activity and drops again on any >~2.5us PE idle gap mid-body (each
    dip costs ~3us). Warm-up matmuls + bridge() fillers keep the PE dense
    through the preamble; deeper buffer pools (opool/tpool/spool) keep it
    dense at chunk boundaries.
  * Preamble DMAs ride the sync queue in split order (w2t/mask halves first)
    so ktab taps 0-4 and the first dg half unlock while the rest streams;
    engines boot (TENSOR_LOAD ~5us) + preamble barrier mean no kernel
    instruction runs before ~6us and no DMA data lands before ~8.7us.
  * The kernel-generator MLP, channel attention and diag-weight build run
    on-device; the attention path (Sigmoid needs a 1.3us act-table load) is
    deferred until after chunk 0's taps so it never gates the body.
  * gpsimd cannot run TensorScalarPtr/TensorTensor (Pool engine check fails
    at codegen); DVE scalar_tensor_tensor runs at 1x (no perf modes) so the
    TS+TT tree beats a fused-STT chain; fp8 (DoubleRow) fails the 2e-2
    error gate (measured 2.4-3.4e-2 vs bf16 6.6e-3).
"""

import os
from collections import deque
from contextlib import ExitStack

import ml_dtypes
import numpy as np

import concourse.bass as bass
import concourse.mybir as mybir
import concourse.tile as tile
from concourse import bacc
from concourse.bass_utils import run_bass_kernel_spmd

AF = mybir.ActivationFunctionType
ALU = mybir.AluOpType
F32 = mybir.dt.float32
BF16 = mybir.dt.bfloat16

B, C, H, W = 8, 128, 128, 128
KK = 3
NT = KK * KK                 # 9 taps
HW = H * W
XOFF = 2                     # interior column offset in the padded layout
WP = W + 4                   # host-padded width (2 left, 2 right)
HP = H + 2                   # host-padded height (1 halo row each side)
R = 32                       # image rows per chunk
NCH = H // R                 # 4 chunks
PE_ROWS_L = [20, 20, 20, 20]  # per-chunk TensorE depthwise rows
FLIP_LAST = True             # chunk 3: DVE rows on top, PE rows at the bottom
TAPS = [(dy, dx) for dy in (-1, 0, 1) for dx in (-1, 0, 1)]  # t = (dy+1)*3+(dx+1)
N_WARM = 3                   # PE warm-up matmuls ahead of the feat matmul

# bf16 weight blob a: w1t | alt | ca1t | ca2t   (small, lands first)
W_W1T, W_ALT, W_CA1T, W_CA2T = 0, 128, 129, 145
WA_COLS = 145 + 128
# bf16 weight blob b: w2t (tap-major: col t*128+c) | cwt
W_W2T, W_CWT = 0, NT * 128
WB_COLS = NT * 128 + 128
WB_SPLIT = 5 * 128           # w2t columns for taps 0-4 (first diag half)
MASK_SPLIT = 5 * 128         # mask columns for the first diag-build half

last_results = None          # BassKernelResults of the most recent run


def _pe_blocks(pe_lo, pe_hi):
    blocks = []
    r = pe_lo
    while r < pe_hi:
        rr = min(4, pe_hi - r)
        blocks.append((r, r + rr))
        r += rr
    return blocks


def _emit(tc, nc, d):
    ctx = d["ctx"]
    singles = ctx.enter_context(tc.tile_pool(name="singles", bufs=1))
    xpool = ctx.enter_context(tc.tile_pool(name="xpool", bufs=3))
    spool = ctx.enter_context(tc.tile_pool(name="spool", bufs=3))
    tpool = ctx.enter_context(tc.tile_pool(name="tpool", bufs=3))
    opool = ctx.enter_context(tc.tile_pool(name="opool", bufs=4))
    pss_pool = ctx.enter_context(tc.tile_pool(name="psum_s", bufs=2, space="PSUM"))
    pso_pool = ctx.enter_context(tc.tile_pool(name="psum_o", bufs=3, space="PSUM"))

    # -- a few PE warm-up matmuls (HAM ramp), then the preamble DMAs on the
    # sync queue in v5's split order: the first wblob_b/mask halves unlock
    # ktab taps 0-4 and the first dg half while the rest streams in --
    warm = singles.tile([128, 512], BF16, name="warm", tag="warm")
    nc.gpsimd.memset(warm, 0.0)
    wps = pso_pool.tile([128, 512], F32, name="wps", tag="pso")
    for _ in range(N_WARM):
        nc.tensor.matmul(wps, lhsT=warm[:, 0:128], rhs=warm, start=True, stop=True)

    wblob_a = singles.tile([128, WA_COLS], BF16, name="wblob_a", tag="wblob_a")
    nc.sync.dma_start(out=wblob_a, in_=d["wblob_a"])
    wblob_b = singles.tile([128, WB_COLS], BF16, name="wblob_b", tag="wblob_b")
    nc.sync.dma_start(out=wblob_b[:, 0:WB_SPLIT], in_=d["wblob_b"][:, 0:WB_SPLIT])
    mask = singles.tile([128, NT * 128], BF16, name="mask", tag="mask")
    nc.sync.dma_start(out=mask[:, 0:MASK_SPLIT], in_=d["mask"][:, 0:MASK_SPLIT])

    x3 = d["xpad"].rearrange("c (h w) -> c h w", w=WP)
    xpf_d = d["xpad"]

    xps = []
    xp0 = xpool.tile([128, R + 2, WP], BF16, name="xp0", tag="xp")
    nc.sync.dma_start(out=xp0[:, 0:6, :], in_=x3[:, 0:6, :])
    nc.sync.dma_start(
        out=wblob_b[:, WB_SPLIT:WB_COLS], in_=d["wblob_b"][:, WB_SPLIT:WB_COLS]
    )
    nc.sync.dma_start(
        out=mask[:, MASK_SPLIT : NT * 128], in_=d["mask"][:, MASK_SPLIT : NT * 128]
    )
    nc.sync.dma_start(out=xp0[:, 6:18, :], in_=x3[:, 6:18, :])
    cb = singles.tile([128, 1], F32, name="cb", tag="cb")
    nc.sync.dma_start(out=cb, in_=d["cb"])
    nc.sync.dma_start(out=xp0[:, 18 : R + 2, :], in_=x3[:, 18 : R + 2, :])
    xps.append(xp0)
    _emit_prologue(tc, nc, d, singles, pss_pool, wblob_a, wblob_b, mask,
                   warm, wps)
    for ci in range(1, NCH):
        y0 = ci * R
        xp = xpool.tile([128, R + 2, WP], BF16, name=f"xp{ci}", tag="xp")
        nc.sync.dma_start(out=xp, in_=x3[:, y0 : y0 + R + 2, :])
        xps.append(xp)

    ktab = d["ktab"]
    dg_all = d["dg_all"]
    cwt = wblob_b[:, W_CWT : W_CWT + 128]
    out_d = d["out"]

    # PE-row-gated tails flush as soon as emitted; DVE-row-gated tails are
    # held until the NEXT chunk's PE blocks are in the queue, so the PE always
    # has independent depthwise work ahead of a tail that may wait on the DVE
    # tree -> scalar relu chain (v11 lost ~5us to those stalls).
    held_dve_tails = []

    for ci in range(NCH):
        y0 = ci * R
        flip = FLIP_LAST and ci == NCH - 1
        pe_rows = PE_ROWS_L[ci]
        dve_rows = R - pe_rows
        dve_lo = 0 if flip else pe_rows
        dve_hi = dve_rows if flip else R
        pe_lo = dve_rows if flip else 0
        pe_hi = R if flip else pe_rows
        xp = xps[ci]
        srelu = spool.tile([128, R * W], BF16, name=f"sr{ci}", tag="sr")

        # -- DVE depthwise rows [dve_lo, dve_hi): 9 TS products into slots of
        # one 4D tile, then a contiguous-halves add tree (4 TTs, big APs) --
        P = tpool.tile([128, 8, dve_rows, W], BF16, name=f"P{ci}", tag="P")
        t8 = tpool.tile([128, dve_rows, W], BF16, name=f"t8{ci}", tag="t8")
        for ti, (dy, dx) in enumerate(TAPS):
            src = xp[:, 1 + dve_lo + dy : 1 + dve_hi + dy,
                     XOFF + dx : XOFF + dx + W]
            out = t8 if ti == 8 else P[:, ti]
            nc.vector.tensor_scalar_mul(out=out, in0=src,
                                        scalar1=ktab[:, ti : ti + 1])
        nc.vector.tensor_tensor(out=P[:, 0:4], in0=P[:, 0:4], in1=P[:, 4:8],
                                op=ALU.add)
        nc.vector.tensor_tensor(out=P[:, 0:2], in0=P[:, 0:2], in1=P[:, 2:4],
                                op=ALU.add)
        nc.vector.tensor_tensor(out=P[:, 0], in0=P[:, 0], in1=P[:, 1],
                                op=ALU.add)
        nc.vector.tensor_tensor(out=P[:, 0], in0=P[:, 0], in1=t8, op=ALU.add)
        nc.scalar.activation(srelu[:, dve_lo * W : dve_hi * W], P[:, 0],
                             AF.Relu)
        if ci == 0:
            d["emit_attention"]()
            attd = d["attd"]

        # -- PE depthwise: rows [pe_lo, pe_hi) in <=4-row PSUM blocks, with
        # the previous chunk's DVE-gated tails interleaved between blocks so
        # their evacs/stores don't burst on the scalar engine and PSUM --
        for bi, (rs, re) in enumerate(_pe_blocks(pe_lo, pe_hi)):
            rows = re - rs
            pss = pss_pool.tile([128, rows * W], F32, name=f"pss{ci}_{rs}",
                                tag="pss")
            for ti, (dy, dx) in enumerate(TAPS):
                rhs = xp[:, 1 + rs + dy : 1 + re + dy, XOFF + dx : XOFF + dx + W]
                nc.tensor.matmul(
                    pss, lhsT=dg_all[:, ti * 128 : (ti + 1) * 128], rhs=rhs,
                    start=(ti == 0), stop=(ti == NT - 1),
                )
            nc.scalar.activation(srelu[:, rs * W : re * W], pss, AF.Relu)
            if bi in (1, 3) and held_dve_tails:
                held_dve_tails.pop(0)()
        for t in held_dve_tails:
            t()
        held_dve_tails = []

        # -- tails: 8-row pointwise+residual units; pairs share one 16-row
        # osb tile and the pair's second tail fires a single 16-row store --
        last = ci == NCH - 1
        osb_pair = {}
        for ti_, tr in enumerate(range(0, R, 8)):
            dve_gated = (tr + 8 > dve_lo) and (tr < dve_hi)
            evac_vec = last and (ti_ % 2 == 0)
            t = _make_tail(nc, pso_pool, opool, xp, srelu, cwt, attd,
                           cb, out_d, ci, tr, y0, evac_vec, osb_pair)
            if dve_gated and not last:
                held_dve_tails.append(t)
            else:
                t()
    for t in held_dve_tails:
        t()


def _emit_prologue(tc, nc, d, singles, pss_pool, wblob_a, wblob_b, mask,
                   warm, wps):
    def bridge(n):
        # fill PE-idle dependency stalls so the HAM clock never re-throttles
        for _ in range(n):
            nc.tensor.matmul(wps, lhsT=warm[:, 0:128], rhs=warm,
                             start=True, stop=True)

    alt = wblob_a[:, W_ALT : W_ALT + 1]
    w1t = wblob_a[:, W_W1T : W_W1T + 128]
    ca1t = wblob_a[:, W_CA1T : W_CA1T + 16]
    ca2t = wblob_a[0:16, W_CA2T : W_CA2T + 128]
    w2t = wblob_b[:, W_W2T : W_W2T + NT * 128]

    def leaky(name, psum_src, parts, dt=F32):
        tmp = singles.tile([parts, 1], F32, name=f"{name}_t", tag=f"{name}_t")
        nc.vector.tensor_scalar_mul(out=tmp, in0=psum_src, scalar1=0.1)
        res = singles.tile([parts, 1], dt, name=name, tag=name)
        nc.vector.tensor_tensor(out=res, in0=tmp, in1=psum_src, op=ALU.max)
        return res

    # ---- kernel-generator MLP (all bf16 matmuls) ----
    feat_ps = pss_pool.tile([128, 1], F32, name="feat_ps", tag="pss")
    nc.tensor.matmul(feat_ps, lhsT=w1t, rhs=alt, start=True, stop=True)
    bridge(2)
    feat = leaky("feat", feat_ps, 128, dt=BF16)

    ktab_ps = pss_pool.tile([128, NT], F32, name="ktab_ps", tag="pss")
    for t in range(5):
        nc.tensor.matmul(
            ktab_ps[:, t : t + 1], lhsT=w2t[:, t * 128 : (t + 1) * 128],
            rhs=feat, start=True, stop=True
        )
    bridge(1)
    for t in range(5, NT):
        nc.tensor.matmul(
            ktab_ps[:, t : t + 1], lhsT=w2t[:, t * 128 : (t + 1) * 128],
            rhs=feat, start=True, stop=True
        )
    bridge(2)
    # SBUF copy (DVE tap scalar source) on the vector engine, ahead of the
    # chunk-0 taps in its queue; the dg build reads ktab_ps straight from PSUM.
    ktab = singles.tile([128, NT], F32, name="ktab", tag="ktab")
    nc.vector.tensor_scalar_add(out=ktab, in0=ktab_ps, scalar1=0.0)

    # ---- diag weights straight from PSUM: dg[:, t*128+j] = I[p,j]*ktab[p,t]
    dg_all = singles.tile([128, NT * 128], BF16, name="dg_all", tag="dg_all")
    ktab_b = ktab_ps.unsqueeze(2).broadcast_to([128, NT, 128])
    mask3 = mask.rearrange("p (t c) -> p t c", t=NT)
    dg3 = dg_all.rearrange("p (t c) -> p t c", t=NT)
    nc.vector.tensor_tensor(
        out=dg3[:, 0:5, :], in0=mask3[:, 0:5, :], in1=ktab_b[:, 0:5, :],
        op=ALU.mult,
    )
    nc.vector.tensor_tensor(
        out=dg3[:, 5:NT, :], in0=mask3[:, 5:NT, :], in1=ktab_b[:, 5:NT, :],
        op=ALU.mult,
    )

    # keep the PE busy across the preamble->body handoff: an idle epoch here
    # makes the HAM down-throttle right as the first depthwise blocks issue
    # (the dg chain completes ~15us: boot+barrier+DMA flow are fixed costs)
    bridge(12)
    d["ktab"] = ktab
    d["dg_all"] = dg_all

    def emit_attention():
        # deferred until after chunk 0's tap emission: the Sigmoid act-table
        # load (~1.3us on the scalar engine) and the attd build must not gate
        # the first DVE taps; attd is first needed by the chunk-0 tails.
        a1_ps = pss_pool.tile([16, 1], F32, name="a1_ps", tag="pss")
        nc.tensor.matmul(a1_ps, lhsT=ca1t, rhs=alt, start=True, stop=True)
        a1 = leaky("a1", a1_ps, 16, dt=BF16)
        att_ps = pss_pool.tile([128, 1], F32, name="att_ps", tag="pss")
        nc.tensor.matmul(att_ps, lhsT=ca2t, rhs=a1, start=True, stop=True)
        attv = singles.tile([128, 1], F32, name="attv", tag="attv")
        nc.scalar.activation(attv, att_ps, AF.Sigmoid)
        attd = singles.tile([128, 128], BF16, name="attd", tag="attd")
        nc.vector.tensor_tensor(
            out=attd, in0=mask[:, 0:128], in1=attv.broadcast_to([128, 128]),
            op=ALU.mult,
        )
        d["attd"] = attd
    d["emit_attention"] = emit_attention


def _make_tail(nc, pso_pool, opool, xp, srelu, cwt, attd, cb, out_d, ci, tr,
               y0, evac_vec=False, osb_pair=None):
    """Two 4-row pointwise+residual groups into one 2-bank PSUM tile, biased
    bf16 evac into half of a shared 16-row osb; the pair's second tail
    stores all 16 rows in one DMA (bigger descriptors, half the triggers)."""

    def tail():
        pk = tr // 16
        if pk in osb_pair:
            osb16 = osb_pair.pop(pk)
            second = True
        else:
            osb16 = opool.tile([128, 16 * W], BF16, name=f"ob{ci}_{pk}",
                               tag="ob")
            osb_pair[pk] = osb16
            second = False
        osb = osb16[:, (tr % 16) * W : (tr % 16 + 8) * W]
        pso = pso_pool.tile([128, 8 * W], F32, name=f"pso{ci}_{tr}", tag="pso")
        for h, r0 in enumerate((tr, tr + 4)):
            half = pso[:, h * 4 * W : (h + 1) * 4 * W]
            nc.tensor.matmul(half, lhsT=cwt, rhs=srelu[:, r0 * W : (r0 + 4) * W],
                             start=True, stop=False)
            nc.tensor.matmul(
                half, lhsT=attd, rhs=xp[:, 1 + r0 : 1 + r0 + 4, XOFF : XOFF + W],
                start=False, stop=True,
            )
        if evac_vec:
            nc.vector.tensor_scalar_add(out=osb, in0=pso, scalar1=cb)
        else:
            nc.scalar.activation(osb, pso, AF.Identity, bias=cb)
        if second:
            nc.gpsimd.dma_start(
                out=out_d[:, (y0 + pk * 16) * W : (y0 + pk * 16 + 16) * W],
                in_=osb16,
            )

    return tail


def build_module():
    nc = bacc.Bacc(
        "TRN2",
        target_bir_lowering=False,
        debug=False,
        enable_asserts=False,
        num_devices=B,
    )
    d = {
        "xpad": nc.dram_tensor("xpad", [C, HP * WP], BF16, kind="ExternalInput").ap(),
        "wblob_a": nc.dram_tensor("wblob_a", [128, WA_COLS], BF16, kind="ExternalInput").ap(),
        "wblob_b": nc.dram_tensor("wblob_b", [128, WB_COLS], BF16, kind="ExternalInput").ap(),
        "cb": nc.dram_tensor("cb", [C, 1], F32, kind="ExternalInput").ap(),
        "mask": nc.dram_tensor("mask", [128, NT * 128], BF16, kind="ExternalInput").ap(),
        "out": nc.dram_tensor("out", [C, HW], BF16, kind="ExternalOutput").ap(),
    }
    with tile.TileContext(nc) as tc:
        with ExitStack() as ctx:
            d["ctx"] = ctx
            _emit(tc, nc, d)
    nc.finalize()
    return nc


_module_cache = None


def _get_module():
    global _module_cache
    if _module_cache is None:
        _module_cache = build_module()
    return _module_cache


def make_in_maps(x, altitude, W1, W2, conv_w, conv_b, ca_w1, ca_w2):
    f = np.float32
    bf = ml_dtypes.bfloat16
    x = np.asarray(x, dtype=f)
    altitude = np.asarray(altitude, dtype=f)
    xpad = np.zeros((B, C, HP, WP), dtype=f)
    xpad[:, :, 1 : H + 1, XOFF : XOFF + W] = x
    xq = np.ascontiguousarray(xpad.astype(bf).reshape(B, C, HP * WP))

    wblob_shared = np.zeros((128, WA_COLS), dtype=bf)
    wblob_shared[:, W_W1T : W_W1T + 128] = np.asarray(W1, dtype=f).T.astype(bf)
    wblob_shared[:, W_CA1T : W_CA1T + 16] = np.asarray(ca_w1, dtype=f).T.astype(bf)
    wblob_shared[0:16, W_CA2T : W_CA2T + 128] = np.asarray(
        ca_w2, dtype=f
    ).T.astype(bf)
    wblob_b = np.zeros((128, WB_COLS), dtype=bf)
    w2tr = (
        np.asarray(W2, dtype=f).T.reshape(128, 128, NT)
        .transpose(0, 2, 1).reshape(128, NT * 128)
    )
    wblob_b[:, W_W2T : W_W2T + NT * 128] = w2tr.astype(bf)
    wblob_b[:, W_CWT : W_CWT + 128] = np.asarray(conv_w, dtype=f).T.astype(bf)
    wblob_b = np.ascontiguousarray(wblob_b)

    cb_arr = np.ascontiguousarray(np.asarray(conv_b, dtype=f).reshape(C, 1))
    mask_arr = np.ascontiguousarray(
        np.tile(np.eye(128, dtype=f), (1, NT)).astype(bf)
    )

    maps = []
    for bb in range(B):
        wblob_a = wblob_shared.copy()
        wblob_a[:, W_ALT] = altitude[bb].astype(bf)
        maps.append({"xpad": xq[bb], "wblob_a": np.ascontiguousarray(wblob_a),
                     "wblob_b": wblob_b, "cb": cb_arr, "mask": mask_arr})
    return maps


def kernel(x, altitude, W1, W2, conv_w, conv_b, ca_w1, ca_w2):
    global last_results
    in_maps = make_in_maps(x, altitude, W1, W2, conv_w, conv_b, ca_w1, ca_w2)
    nc = _get_module()
    trace = os.environ.get("KERNEL_TRACE", "0") == "1"
    last_results = run_bass_kernel_spmd(
        nc, in_maps, core_ids=list(range(B)), trace=trace
    )
    out = np.stack(
        [
            last_results.results[bb]["out"].astype(np.float32).reshape(C, H, W)
            for bb in range(B)
        ]
    )
    return out


# revision 19
# speedup vs baseline: 1.0584x; 1.0024x over previous
"""Trainium2 Bass kernel for nn_DA_conv: per-sample generated depthwise 3x3 conv
-> relu -> 1x1 pointwise conv (+bias) -> + x * channel_attention(altitude).

Data-parallel over batch: 8 samples -> 8 NeuronCores, weights replicated.

v18 design (trace-driven; v5 baseline 80.8us -> 77.7us):
  * Rows PE 80 / DVE 48. Measured rates: PE depthwise row (9 diag bf16
    matmuls, N=512 issue interval 215ns) 484ns; DVE row ~1.04us.
  * DVE 4x tensor_scalar mode requires an even row count per op (free size
    multiple of 256 elements); odd-element column offsets are fine (the old
    xb1 shifted-copy DMA was unnecessary - all taps read xp directly).
  * DVE tap products write slots of one 4D tile [128,8,rows,W]; the add tree
    is 4 contiguous-halves tensor_tensors with big APs (fewer instructions,
    ~0.6us/chunk less DVE time than a pairwise tree over 9 tiles).
  * Tails (pointwise cwt + diag(att) residual into 2 PSUM banks, biased bf16
    evac) are split by dependency: PE-row tails flush immediately; DVE-row-
    gated tails are held and interleaved between the NEXT chunk's depthwise
    blocks, so the PE always has independent work queued ahead of a tail
    that may wait on the DVE tree -> relu chain.
  * Tail pairs share a 16-row osb tile; one 16-row store per pair (bigger
    DMA descriptors, half the SWDGE triggers, less osb recycle pressure -
    store completion gates osb reuse and stalled the PE in v15/earlier).
  * Chunk 3 is flipped (DVE rows on top) so the final tails consume
    PE-produced rows; its evacs alternate vector/scalar to drain faster.
  * HAM clock: k=4/8 at boot, lifts after ~1-2 epochs (3.4us) of sustained
    PE activity and drops again on any >~2.5us PE idle gap mid-body (each
    dip costs ~3us). Warm-up matmuls + bridge() fillers keep the PE dense
    through the preamble; deeper buffer pools (opool/tpool/spool) keep it
    dense at chunk boundaries.
  * Preamble DMAs ride the sync queue in split order (w2t/mask halves first)
    so ktab taps 0-4 and the first dg half unlock while the rest streams;
    engines boot (TENSOR_LOAD ~5us) + preamble barrier mean no kernel
    instruction runs before ~6us and no DMA data lands before ~8.7us.
  * The kernel-generator MLP, channel attention and diag-weight build run
    on-device; the attention path (Sigmoid needs a 1.3us act-table load) is
    deferred until after chunk 0's taps so it never gates the body.
  * gpsimd cannot run TensorScalarPtr/TensorTensor (Pool engine check fails
    at codegen); DVE scalar_tensor_tensor runs at 1x (no perf modes) so the
    TS+TT tree beats a fused-STT chain; fp8 (DoubleRow) fails the 2e-2
    error gate (measured 2.4-3.4e-2 vs bf16 6.6e-3).
"""

import os
from collections import deque
from contextlib import ExitStack

import ml_dtypes
import numpy as np

import concourse.bass as bass
import concourse.mybir as mybir
import concourse.tile as tile
from concourse import bacc
from concourse.bass_utils import run_bass_kernel_spmd

AF = mybir.ActivationFunctionType
ALU = mybir.AluOpType
F32 = mybir.dt.float32
BF16 = mybir.dt.bfloat16

B, C, H, W = 8, 128, 128, 128
KK = 3
NT = KK * KK                 # 9 taps
HW = H * W
XOFF = 2                     # interior column offset in the padded layout
WP = W + 4                   # host-padded width (2 left, 2 right)
HP = H + 2                   # host-padded height (1 halo row each side)
R = 32                       # image rows per chunk
NCH = H // R                 # 4 chunks
PE_ROWS_L = [20, 20, 20, 20]  # per-chunk TensorE depthwise rows
FLIP_LAST = True             # chunk 3: DVE rows on top, PE rows at the bottom
TAPS = [(dy, dx) for dy in (-1, 0, 1) for dx in (-1, 0, 1)]  # t = (dy+1)*3+(dx+1)
N_WARM = 3                   # PE warm-up matmuls ahead of the feat matmul

# bf16 weight blob a: w1t | alt | ca1t | ca2t   (small, lands first)
W_W1T, W_ALT, W_CA1T, W_CA2T = 0, 128, 129, 145
WA_COLS = 145 + 128
# bf16 weight blob b: w2t (tap-major: col t*128+c) | cwt
W_W2T, W_CWT = 0, NT * 128
WB_COLS = NT * 128 + 128
WB_SPLIT = 5 * 128           # w2t columns for taps 0-4 (first diag half)
MASK_SPLIT = 5 * 128         # mask columns for the first diag-build half

last_results = None          # BassKernelResults of the most recent run


def _pe_blocks(pe_lo, pe_hi):
    blocks = []
    r = pe_lo
    while r < pe_hi:
        rr = min(4, pe_hi - r)
        blocks.append((r, r + rr))
        r += rr
    return blocks


def _emit(tc, nc, d):
    ctx = d["ctx"]
    singles = ctx.enter_context(tc.tile_pool(name="singles", bufs=1))
    xpool = ctx.enter_context(tc.tile_pool(name="xpool", bufs=3))
    spool = ctx.enter_context(tc.tile_pool(name="spool", bufs=3))
    tpool = ctx.enter_context(tc.tile_pool(name="tpool", bufs=3))
    opool = ctx.enter_context(tc.tile_pool(name="opool", bufs=4))
    pss_pool = ctx.enter_context(tc.tile_pool(name="psum_s", bufs=2, space="PSUM"))
    pso_pool = ctx.enter_context(tc.tile_pool(name="psum_o", bufs=3, space="PSUM"))

    # -- a few PE warm-up matmuls (HAM ramp), then the preamble DMAs on the
    # sync queue in v5's split order: the first wblob_b/mask halves unlock
    # ktab taps 0-4 and the first dg half while the rest streams in --
    warm = singles.tile([128, 512], BF16, name="warm", tag="warm")
    nc.gpsimd.memset(warm, 0.0)
    wps = pso_pool.tile([128, 512], F32, name="wps", tag="pso")
    for _ in range(N_WARM):
        nc.tensor.matmul(wps, lhsT=warm[:, 0:128], rhs=warm, start=True, stop=True)

    wblob_a = singles.tile([128, WA_COLS], BF16, name="wblob_a", tag="wblob_a")
    nc.sync.dma_start(out=wblob_a, in_=d["wblob_a"])
    wblob_b = singles.tile([128, WB_COLS], BF16, name="wblob_b", tag="wblob_b")
    nc.sync.dma_start(out=wblob_b[:, 0:WB_SPLIT], in_=d["wblob_b"][:, 0:WB_SPLIT])
    mask = singles.tile([128, NT * 128], BF16, name="mask", tag="mask")
    nc.sync.dma_start(out=mask[:, 0:MASK_SPLIT], in_=d["mask"][:, 0:MASK_SPLIT])

    x3 = d["xpad"].rearrange("c (h w) -> c h w", w=WP)
    xpf_d = d["xpad"]

    xps = []
    xp0 = xpool.tile([128, R + 2, WP], BF16, name="xp0", tag="xp")
    nc.sync.dma_start(out=xp0[:, 0:6, :], in_=x3[:, 0:6, :])
    nc.sync.dma_start(
        out=wblob_b[:, WB_SPLIT:WB_COLS], in_=d["wblob_b"][:, WB_SPLIT:WB_COLS]
    )
    nc.sync.dma_start(
        out=mask[:, MASK_SPLIT : NT * 128], in_=d["mask"][:, MASK_SPLIT : NT * 128]
    )
    nc.sync.dma_start(out=xp0[:, 6:18, :], in_=x3[:, 6:18, :])
    cb = singles.tile([128, 1], F32, name="cb", tag="cb")
    nc.sync.dma_start(out=cb, in_=d["cb"])
    nc.sync.dma_start(out=xp0[:, 18 : R + 2, :], in_=x3[:, 18 : R + 2, :])
    xps.append(xp0)
    _emit_prologue(tc, nc, d, singles, pss_pool, wblob_a, wblob_b, mask,
                   warm, wps)
    for ci in range(1, NCH):
        y0 = ci * R
        xp = xpool.tile([128, R + 2, WP], BF16, name=f"xp{ci}", tag="xp")
        nc.sync.dma_start(out=xp, in_=x3[:, y0 : y0 + R + 2, :])
        xps.append(xp)

    ktab = d["ktab"]
    dg_all = d["dg_all"]
    cwt = wblob_b[:, W_CWT : W_CWT + 128]
    out_d = d["out"]

    # PE-row-gated tails flush as soon as emitted; DVE-row-gated tails are
    # held until the NEXT chunk's PE blocks are in the queue, so the PE always
    # has independent depthwise work ahead of a tail that may wait on the DVE
    # tree -> scalar relu chain (v11 lost ~5us to those stalls).
    held_dve_tails = []

    for ci in range(NCH):
        y0 = ci * R
        flip = FLIP_LAST and ci == NCH - 1
        pe_rows = PE_ROWS_L[ci]
        dve_rows = R - pe_rows
        dve_lo = 0 if flip else pe_rows
        dve_hi = dve_rows if flip else R
        pe_lo = dve_rows if flip else 0
        pe_hi = R if flip else pe_rows
        xp = xps[ci]
        srelu = spool.tile([128, R * W], BF16, name=f"sr{ci}", tag="sr")

        # -- DVE depthwise rows [dve_lo, dve_hi): 9 TS products into slots of
        # one 4D tile, then a contiguous-halves add tree (4 TTs, big APs) --
        P = tpool.tile([128, 8, dve_rows, W], BF16, name=f"P{ci}", tag="P")
        t8 = tpool.tile([128, dve_rows, W], BF16, name=f"t8{ci}", tag="t8")
        for ti, (dy, dx) in enumerate(TAPS):
            src = xp[:, 1 + dve_lo + dy : 1 + dve_hi + dy,
                     XOFF + dx : XOFF + dx + W]
            out = t8 if ti == 8 else P[:, ti]
            nc.vector.tensor_scalar_mul(out=out, in0=src,
                                        scalar1=ktab[:, ti : ti + 1])
        nc.vector.tensor_tensor(out=P[:, 0:4], in0=P[:, 0:4], in1=P[:, 4:8],
                                op=ALU.add)
        nc.vector.tensor_tensor(out=P[:, 0:2], in0=P[:, 0:2], in1=P[:, 2:4],
                                op=ALU.add)
        nc.vector.tensor_tensor(out=P[:, 0], in0=P[:, 0], in1=P[:, 1],
                                op=ALU.add)
        nc.vector.tensor_tensor(out=P[:, 0], in0=P[:, 0], in1=t8, op=ALU.add)
        nc.scalar.activation(srelu[:, dve_lo * W : dve_hi * W], P[:, 0],
                             AF.Relu)
        if ci == 0:
            d["emit_attention"]()
            attd = d["attd"]

        # -- PE depthwise: rows [pe_lo, pe_hi) in <=4-row PSUM blocks, with
        # the previous chunk's DVE-gated tails interleaved between blocks so
        # their evacs/stores don't burst on the scalar engine and PSUM --
        for bi, (rs, re) in enumerate(_pe_blocks(pe_lo, pe_hi)):
            rows = re - rs
            pss = pss_pool.tile([128, rows * W], F32, name=f"pss{ci}_{rs}",
                                tag="pss")
            for ti, (dy, dx) in enumerate(TAPS):
                rhs = xp[:, 1 + rs + dy : 1 + re + dy, XOFF + dx : XOFF + dx + W]
                nc.tensor.matmul(
                    pss, lhsT=dg_all[:, ti * 128 : (ti + 1) * 128], rhs=rhs,
                    start=(ti == 0), stop=(ti == NT - 1),
                )
            nc.scalar.activation(srelu[:, rs * W : re * W], pss, AF.Relu)
            if bi in (1, 3) and held_dve_tails:
                held_dve_tails.pop(0)()
        for t in held_dve_tails:
            t()
        held_dve_tails = []

        # -- tails: 8-row pointwise+residual units; pairs share one 16-row
        # osb tile and the pair's second tail fires a single 16-row store --
        last = ci == NCH - 1
        osb_pair = {}
        for ti_, tr in enumerate(range(0, R, 8)):
            dve_gated = (tr + 8 > dve_lo) and (tr < dve_hi)
            evac_vec = last and (ti_ % 2 == 0)
            t = _make_tail(nc, pso_pool, opool, xp, srelu, cwt, attd,
                           cb, out_d, ci, tr, y0, evac_vec, osb_pair,
                           solo_store=last)
            if dve_gated and not last:
                held_dve_tails.append(t)
            else:
                t()
    for t in held_dve_tails:
        t()


def _emit_prologue(tc, nc, d, singles, pss_pool, wblob_a, wblob_b, mask,
                   warm, wps):
    def bridge(n):
        # fill PE-idle dependency stalls so the HAM clock never re-throttles
        for _ in range(n):
            nc.tensor.matmul(wps, lhsT=warm[:, 0:128], rhs=warm,
                             start=True, stop=True)

    alt = wblob_a[:, W_ALT : W_ALT + 1]
    w1t = wblob_a[:, W_W1T : W_W1T + 128]
    ca1t = wblob_a[:, W_CA1T : W_CA1T + 16]
    ca2t = wblob_a[0:16, W_CA2T : W_CA2T + 128]
    w2t = wblob_b[:, W_W2T : W_W2T + NT * 128]

    def leaky(name, psum_src, parts, dt=F32):
        tmp = singles.tile([parts, 1], F32, name=f"{name}_t", tag=f"{name}_t")
        nc.vector.tensor_scalar_mul(out=tmp, in0=psum_src, scalar1=0.1)
        res = singles.tile([parts, 1], dt, name=name, tag=name)
        nc.vector.tensor_tensor(out=res, in0=tmp, in1=psum_src, op=ALU.max)
        return res

    # ---- kernel-generator MLP (all bf16 matmuls) ----
    feat_ps = pss_pool.tile([128, 1], F32, name="feat_ps", tag="pss")
    nc.tensor.matmul(feat_ps, lhsT=w1t, rhs=alt, start=True, stop=True)
    bridge(2)
    feat = leaky("feat", feat_ps, 128, dt=BF16)

    ktab_ps = pss_pool.tile([128, NT], F32, name="ktab_ps", tag="pss")
    for t in range(5):
        nc.tensor.matmul(
            ktab_ps[:, t : t + 1], lhsT=w2t[:, t * 128 : (t + 1) * 128],
            rhs=feat, start=True, stop=True
        )
    bridge(1)
    for t in range(5, NT):
        nc.tensor.matmul(
            ktab_ps[:, t : t + 1], lhsT=w2t[:, t * 128 : (t + 1) * 128],
            rhs=feat, start=True, stop=True
        )
    bridge(2)
    # SBUF copy (DVE tap scalar source) on the vector engine, ahead of the
    # chunk-0 taps in its queue; the dg build reads ktab_ps straight from PSUM.
    ktab = singles.tile([128, NT], F32, name="ktab", tag="ktab")
    nc.vector.tensor_scalar_add(out=ktab, in0=ktab_ps, scalar1=0.0)

    # ---- diag weights straight from PSUM: dg[:, t*128+j] = I[p,j]*ktab[p,t]
    dg_all = singles.tile([128, NT * 128], BF16, name="dg_all", tag="dg_all")
    ktab_b = ktab_ps.unsqueeze(2).broadcast_to([128, NT, 128])
    mask3 = mask.rearrange("p (t c) -> p t c", t=NT)
    dg3 = dg_all.rearrange("p (t c) -> p t c", t=NT)
    nc.vector.tensor_tensor(
        out=dg3[:, 0:5, :], in0=mask3[:, 0:5, :], in1=ktab_b[:, 0:5, :],
        op=ALU.mult,
    )
    nc.vector.tensor_tensor(
        out=dg3[:, 5:NT, :], in0=mask3[:, 5:NT, :], in1=ktab_b[:, 5:NT, :],
        op=ALU.mult,
    )

    # keep the PE busy across the preamble->body handoff: an idle epoch here
    # makes the HAM down-throttle right as the first depthwise blocks issue
    # (the dg chain completes ~15us: boot+barrier+DMA flow are fixed costs)
    bridge(12)
    d["ktab"] = ktab
    d["dg_all"] = dg_all

    def emit_attention():
        # deferred until after chunk 0's tap emission: the Sigmoid act-table
        # load (~1.3us on the scalar engine) and the attd build must not gate
        # the first DVE taps; attd is first needed by the chunk-0 tails.
        a1_ps = pss_pool.tile([16, 1], F32, name="a1_ps", tag="pss")
        nc.tensor.matmul(a1_ps, lhsT=ca1t, rhs=alt, start=True, stop=True)
        a1 = leaky("a1", a1_ps, 16, dt=BF16)
        att_ps = pss_pool.tile([128, 1], F32, name="att_ps", tag="pss")
        nc.tensor.matmul(att_ps, lhsT=ca2t, rhs=a1, start=True, stop=True)
        attv = singles.tile([128, 1], F32, name="attv", tag="attv")
        nc.scalar.activation(attv, att_ps, AF.Sigmoid)
        attd = singles.tile([128, 128], BF16, name="attd", tag="attd")
        nc.vector.tensor_tensor(
            out=attd, in0=mask[:, 0:128], in1=attv.broadcast_to([128, 128]),
            op=ALU.mult,
        )
        d["attd"] = attd
    d["emit_attention"] = emit_attention


def _make_tail(nc, pso_pool, opool, xp, srelu, cwt, attd, cb, out_d, ci, tr,
               y0, evac_vec=False, osb_pair=None, solo_store=False):
    """Two 4-row pointwise+residual groups into one 2-bank PSUM tile, biased
    bf16 evac into half of a shared 16-row osb; the pair's second tail
    stores all 16 rows in one DMA (bigger descriptors, half the triggers)."""

    def tail():
        pk = tr // 16
        if pk in osb_pair:
            osb16 = osb_pair.pop(pk)
            second = True
        else:
            osb16 = opool.tile([128, 16 * W], BF16, name=f"ob{ci}_{pk}",
                               tag="ob")
            osb_pair[pk] = osb16
            second = False
        osb = osb16[:, (tr % 16) * W : (tr % 16 + 8) * W]
        pso = pso_pool.tile([128, 8 * W], F32, name=f"pso{ci}_{tr}", tag="pso")
        for h, r0 in enumerate((tr, tr + 4)):
            half = pso[:, h * 4 * W : (h + 1) * 4 * W]
            nc.tensor.matmul(half, lhsT=cwt, rhs=srelu[:, r0 * W : (r0 + 4) * W],
                             start=True, stop=False)
            nc.tensor.matmul(
                half, lhsT=attd, rhs=xp[:, 1 + r0 : 1 + r0 + 4, XOFF : XOFF + W],
                start=False, stop=True,
            )
        if evac_vec:
            nc.vector.tensor_scalar_add(out=osb, in0=pso, scalar1=cb)
        else:
            nc.scalar.activation(osb, pso, AF.Identity, bias=cb)
        if solo_store:
            # last chunk: store each 8-row half as soon as its evac lands so
            # the final DMA isn't gated on the pair's second evac
            nc.gpsimd.dma_start(
                out=out_d[:, (y0 + tr) * W : (y0 + tr + 8) * W], in_=osb
            )
        elif second:
            nc.gpsimd.dma_start(
                out=out_d[:, (y0 + pk * 16) * W : (y0 + pk * 16 + 16) * W],
                in_=osb16,
            )

    return tail


def build_module():
    nc = bacc.Bacc(
        "TRN2",
        target_bir_lowering=False,
        debug=False,
        enable_asserts=False,
        num_devices=B,
    )
    d = {
        "xpad": nc.dram_tensor("xpad", [C, HP * WP], BF16, kind="ExternalInput").ap(),
        "wblob_a": nc.dram_tensor("wblob_a", [128, WA_COLS], BF16, kind="ExternalInput").ap(),
        "wblob_b": nc.dram_tensor("wblob_b", [128, WB_COLS], BF16, kind="ExternalInput").ap(),
        "cb": nc.dram_tensor("cb", [C, 1], F32, kind="ExternalInput").ap(),
        "mask": nc.dram_tensor("mask", [128, NT * 128], BF16, kind="ExternalInput").ap(),
        "out": nc.dram_tensor("out", [C, HW], BF16, kind="ExternalOutput").ap(),
    }
    with tile.TileContext(nc) as tc:
        with ExitStack() as ctx:
            d["ctx"] = ctx
            _emit(tc, nc, d)
    nc.finalize()
    return nc


_module_cache = None


def _get_module():
    global _module_cache
    if _module_cache is None:
        _module_cache = build_module()
    return _module_cache


def make_in_maps(x, altitude, W1, W2, conv_w, conv_b, ca_w1, ca_w2):
    f = np.float32
    bf = ml_dtypes.bfloat16
    x = np.asarray(x, dtype=f)
    altitude = np.asarray(altitude, dtype=f)
    xpad = np.zeros((B, C, HP, WP), dtype=f)
    xpad[:, :, 1 : H + 1, XOFF : XOFF + W] = x
    xq = np.ascontiguousarray(xpad.astype(bf).reshape(B, C, HP * WP))

    wblob_shared = np.zeros((128, WA_COLS), dtype=bf)
    wblob_shared[:, W_W1T : W_W1T + 128] = np.asarray(W1, dtype=f).T.astype(bf)
    wblob_shared[:, W_CA1T : W_CA1T + 16] = np.asarray(ca_w1, dtype=f).T.astype(bf)
    wblob_shared[0:16, W_CA2T : W_CA2T + 128] = np.asarray(
        ca_w2, dtype=f
    ).T.astype(bf)
    wblob_b = np.zeros((128, WB_COLS), dtype=bf)
    w2tr = (
        np.asarray(W2, dtype=f).T.reshape(128, 128, NT)
        .transpose(0, 2, 1).reshape(128, NT * 128)
    )
    wblob_b[:, W_W2T : W_W2T + NT * 128] = w2tr.astype(bf)
    wblob_b[:, W_CWT : W_CWT + 128] = np.asarray(conv_w, dtype=f).T.astype(bf)
    wblob_b = np.ascontiguousarray(wblob_b)

    cb_arr = np.ascontiguousarray(np.asarray(conv_b, dtype=f).reshape(C, 1))
    mask_arr = np.ascontiguousarray(
        np.tile(np.eye(128, dtype=f), (1, NT)).astype(bf)
    )

    maps = []
    for bb in range(B):
        wblob_a = wblob_shared.copy()
        wblob_a[:, W_ALT] = altitude[bb].astype(bf)
        maps.append({"xpad": xq[bb], "wblob_a": np.ascontiguousarray(wblob_a),
                     "wblob_b": wblob_b, "cb": cb_arr, "mask": mask_arr})
    return maps


def kernel(x, altitude, W1, W2, conv_w, conv_b, ca_w1, ca_w2):
    global last_results
    in_maps = make_in_maps(x, altitude, W1, W2, conv_w, conv_b, ca_w1, ca_w2)
    nc = _get_module()
    trace = os.environ.get("KERNEL_TRACE", "0") == "1"
    last_results = run_bass_kernel_spmd(
        nc, in_maps, core_ids=list(range(B)), trace=trace
    )
    out = np.stack(
        [
            last_results.results[bb]["out"].astype(np.float32).reshape(C, H, W)
            for bb in range(B)
        ]
    )
    return out
